# revision 18
# baseline (speedup 1.0000x reference)
"""Trainium2 Bass kernel for nn_AMIPRouterInference (gnn_message_passing).

v2: flat pool (no phase barriers), A-phase overlapped with score AllReduce +
softmax, in-place m1b->hid buffer merge, FD=512 vector ops, partition-parallel
softmax, SBUF-resident combine weights, NRS=4 reduce-scatter.
"""

import os
import numpy as np

NCORES = 8

_GRAPH_CACHE = {}
LAST_RESULT = None  # BassKernelResults of the most recent device run


def build_tables(m_idx, u_idx, r, pmax):
    M = len(m_idx)
    dists = np.abs(m_idx[:, None].astype(np.int64) - u_idx[None, :].astype(np.int64))
    adj = (dists > 0) & (dists <= r)
    pair_m, pair_u = np.nonzero(adj)  # row-major == jnp.nonzero order
    pair_m = pair_m[:pmax]
    pair_u = pair_u[:pmax]
    offs = np.unique(pair_u - pair_m).astype(np.int64)
    J = len(offs)
    valid = np.zeros((J, M), dtype=np.float32)
    for j, d in enumerate(offs):
        valid[j, pair_m[(pair_u - pair_m) == d]] = 1.0
    return offs, valid


def build_graph(cfg):
    import contextlib
    import concourse.mybir as mybir
    import concourse.tile as tile
    from concourse import bacc

    D, H, M, U, DP, K = cfg["D"], cfg["H"], cfg["M"], cfg["U"], cfg["DP"], cfg["K"]
    NC = cfg["NC"]
    offs = cfg["offs"]
    J = len(offs)
    PAD = cfg["PAD"]
    MCW = cfg["MCW"]            # compute chunk width along M
    NMC = M // MCW
    QCW = cfg["QCW"]            # qk/score-phase chunk width
    NQC = M // QCW
    DB, HB, DPB = D // 128, H // 128, DP // 128
    HGS = 2                     # h-blocks per A/M-phase psum group
    DGS = 4                     # d-blocks per W2-phase psum group
    DSLAB = 4                   # d-blocks per weight DMA slab
    HSLAB = 4                   # h-blocks per W2 weight DMA slab
    RSD = D // NC               # rows of final output per core
    NRS = cfg["NRS"]            # number of reduce-scatter column groups
    RSW = M // NRS
    b2z = cfg.get("b2z", False)
    assert M % MCW == 0 and M % QCW == 0 and M % NRS == 0 and MCW % RSW == 0

    bf16 = mybir.dt.bfloat16
    f32 = mybir.dt.float32
    AF = mybir.ActivationFunctionType
    hid_af = getattr(AF, cfg.get("hid_act", "Gelu"))

    nc = bacc.Bacc(None, target_bir_lowering=False, debug=False)

    # ---------------- DRAM parameters ----------------
    hmT = nc.declare_dram_parameter("hmT", [D, M], bf16, isOutput=False)
    huT = nc.declare_dram_parameter("huT", [D, U], bf16, isOutput=False)
    w1a = nc.declare_dram_parameter("w1a", [D, H], bf16, isOutput=False)
    w1b = nc.declare_dram_parameter("w1b", [D, H], bf16, isOutput=False)
    w2 = nc.declare_dram_parameter("w2", [H, D], bf16, isOutput=False)
    wq = nc.declare_dram_parameter("wq", [D, 128], bf16, isOutput=False)
    wk = nc.declare_dram_parameter("wk", [D, 128], bf16, isOutput=False)
    wr = nc.declare_dram_parameter("wr", [D, K], bf16, isOutput=False)
    b1c = nc.declare_dram_parameter("b1c", [128, HB], f32, isOutput=False)
    b2r = nc.declare_dram_parameter("b2r", [1, D], bf16, isOutput=False)
    bqc = nc.declare_dram_parameter("bqc", [128, 1], f32, isOutput=False)
    bkc = nc.declare_dram_parameter("bkc", [128, 1], f32, isOutput=False)
    brc = nc.declare_dram_parameter("brc", [128, 1], f32, isOutput=False)
    esel = nc.declare_dram_parameter("esel", [K, 1], bf16, isOutput=False)
    selbc = nc.declare_dram_parameter("selbc", [J, J * 128], bf16,
                                      isOutput=False)
    vmask = nc.declare_dram_parameter("vmask", [J, M], bf16, isOutput=False)
    outp = nc.declare_dram_parameter("out", [RSD, M], bf16, isOutput=True)

    with tile.TileContext(nc) as tc, contextlib.ExitStack() as ctx:
        sb = ctx.enter_context(tc.tile_pool(name="sb", bufs=1))
        ps = ctx.enter_context(tc.tile_pool(name="ps", bufs=1, space="PSUM"))
        dram = ctx.enter_context(tc.tile_pool(name="dram", bufs=1, space="DRAM"))

        def psum_mm(name):
            return ps.tile([128, 512], f32, tag="mm", bufs=6, name=name)

        def psum_row(name):
            return ps.tile([16, 512], f32, tag="row", bufs=2, name=name)

        # ---------------- persistent SBUF tensors ----------------
        ones = sb.tile([128, 128], bf16, name="ones")
        nc.vector.memset(ones[:, :], 1.0)
        ones32 = sb.tile([128, 1], f32, name="ones32")
        nc.vector.memset(ones32[:, :], 1.0)

        b1_sb = sb.tile([128, HB], f32, name="b1_sb")
        nc.sync.dma_start(b1_sb[:, :], b1c[:, :])
        bq_sb = sb.tile([128, 1], f32, name="bq_sb")
        nc.sync.dma_start(bq_sb[:, :], bqc[:, :])
        bk_sb = sb.tile([128, 1], f32, name="bk_sb")
        nc.sync.dma_start(bk_sb[:, :], bkc[:, :])
        br_sb = sb.tile([128, 1], f32, name="br_sb")
        nc.sync.dma_start(br_sb[:, :], brc[:, :])
        esel_sb = sb.tile([K, 1], bf16, name="esel_sb")
        nc.sync.dma_start(esel_sb[:, :], esel[:, :])
        wr_sb = sb.tile([128, DB, K], bf16, name="wr_sb")
        nc.sync.dma_start(
            wr_sb[:, :, :], wr.ap().rearrange("(o p) k -> p o k", p=128)
        )
        if not b2z:
            b2_sb = sb.tile([1, D], bf16, name="b2_sb")
            nc.sync.dma_start(b2_sb[:, :], b2r[:, :])
        vm_sb = sb.tile([J, M], bf16, name="vm_sb")
        nc.sync.dma_start(vm_sb[:, :], vmask[:, :])

        # big persistent tensors
        A1T = sb.tile([128, HB, U + 2 * PAD], bf16, name="A1T")
        for hb in range(HB):
            nc.vector.memset(A1T[:, hb, 0:PAD], 0.0)
            nc.vector.memset(A1T[:, hb, PAD + U: U + 2 * PAD], 0.0)

        def mh_tile(name):
            # holds M1 (post-bias) per chunk, overwritten in place by hid
            return sb.tile([128, HB, MCW], bf16, tag="mh", bufs=2, name=name)

        def hmc_tile(name):
            # streamed h chunks (hu for A phase, hm for M phase)
            return sb.tile([128, DB, MCW], bf16, tag="hmc", bufs=2, name=name)

        def w1t_tile(name):
            return sb.tile([128, DSLAB, HGS * 128], bf16, tag="w1t", bufs=2,
                           name=name)

        def w2t_tile(name):
            return sb.tile([128, HSLAB, DGS * 128], bf16, tag="w2t", bufs=2,
                           name=name)

        def cwb_tile(name):
            return sb.tile([128, J, MCW], bf16, tag="cwb", bufs=2, name=name)

        # qk / softmax phase tiles
        kT_sb = sb.tile([128, U + 2 * PAD], bf16, name="kT_sb")
        nc.vector.memset(kT_sb[:, 0:PAD], 0.0)
        nc.vector.memset(kT_sb[:, PAD + U: U + 2 * PAD], 0.0)
        qT_sb = sb.tile([128, M], bf16, name="qT_sb")
        eg_sb = sb.tile([K, M], bf16, name="eg_sb")
        ej = sb.tile([J, M], bf16, name="ej")
        r0 = sb.tile([1, M], bf16, name="r0")
        g0 = sb.tile([1, M], bf16, name="g0")
        g1 = sb.tile([1, M], bf16, name="g1")
        wbJ = sb.tile([J, M], bf16, name="wbJ")
        cwsum_bf = sb.tile([1, M], bf16, name="cwsum_bf") if not b2z else None
        # one-hot selectors: sel_bc[c, j*128+p] = (c == j), used as matmul lhsT
        # to broadcast row j of a [J, M] tile across 128 partitions
        sel_bc = sb.tile([J, J * 128], bf16, name="sel_bc")
        nc.sync.dma_start(sel_bc[:, :], selbc[:, :])

        sraw_b = dram.tile([J, M], f32, name="sraw_b")
        sred_b = dram.tile(
            [J, M], f32, name="sred_b",
            addr_space="Shared" if NC > 4 else "Local",
        )
        bounce = [
            dram.tile([D, RSW], bf16, name=f"bounce{g}") for g in range(NRS)
        ]
        rsout = [
            dram.tile([RSD, RSW], bf16, name=f"rsout{g}") for g in range(NRS)
        ]

        # ================= phase 1: q/k/gate + raw scores =================
        DBB = 2
        for ch in range(NQC):
            csl = slice(ch * QCW, (ch + 1) * QCW)
            psq = psum_mm("psq")
            psg = psum_row("psg")
            for dbb in range(0, DB, DBB):
                hm_t = sb.tile([128, DBB, QCW], bf16, tag="ht", bufs=2,
                               name="hm_t")
                nc.sync.dma_start(
                    hm_t[:, :, :],
                    hmT[dbb * 128:(dbb + DBB) * 128, csl].rearrange(
                        "(o p) m -> p o m", p=128),
                )
                wq_t = sb.tile([128, DBB, 128], bf16, tag="wt", bufs=2,
                               name="wq_t")
                nc.sync.dma_start(
                    wq_t[:, :, :],
                    wq[dbb * 128:(dbb + DBB) * 128, :].rearrange(
                        "(o p) m -> p o m", p=128),
                )
                for i in range(DBB):
                    db = dbb + i
                    st, sp = db == 0, db == DB - 1
                    nc.tensor.matmul(
                        psq[:, :QCW], wq_t[:, i, :], hm_t[:, i, :],
                        start=st, stop=sp,
                    )
                    nc.tensor.matmul(
                        psg[:K, :QCW], wr_sb[:, db, :], hm_t[:, i, :],
                        start=st, stop=sp,
                    )
            nc.vector.tensor_scalar_add(
                qT_sb[:, csl], psq[:, :QCW], bq_sb[:, 0:1],
            )
            nc.scalar.activation(
                eg_sb[:, csl], psg[:K, :QCW], AF.Exp,
                bias=br_sb[0:K, 0:1], scale=1.0,
            )
            psk = psum_mm("psk")
            for dbb in range(0, DB, DBB):
                hu_t = sb.tile([128, DBB, QCW], bf16, tag="ht", bufs=2,
                               name="hu_t")
                nc.sync.dma_start(
                    hu_t[:, :, :],
                    huT[dbb * 128:(dbb + DBB) * 128, csl].rearrange(
                        "(o p) m -> p o m", p=128),
                )
                wk_t = sb.tile([128, DBB, 128], bf16, tag="wt", bufs=2,
                               name="wk_t")
                nc.sync.dma_start(
                    wk_t[:, :, :],
                    wk[dbb * 128:(dbb + DBB) * 128, :].rearrange(
                        "(o p) m -> p o m", p=128),
                )
                for i in range(DBB):
                    db = dbb + i
                    nc.tensor.matmul(
                        psk[:, :QCW], wk_t[:, i, :], hu_t[:, i, :],
                        start=db == 0, stop=db == DB - 1,
                    )
            nc.vector.tensor_scalar_add(
                kT_sb[:, PAD + ch * QCW: PAD + (ch + 1) * QCW],
                psk[:, :QCW], bk_sb[:, 0:1],
            )

        for ch in range(NQC):
            for j in range(J):
                off = int(offs[j])
                pss = psum_row(f"pss{j}")
                prod = sb.tile([128, QCW], bf16, tag="prod", bufs=2,
                               name="prod")
                nc.vector.tensor_mul(
                    out=prod[:, :],
                    in0=qT_sb[:, ch * QCW:(ch + 1) * QCW],
                    in1=kT_sb[:, PAD + off + ch * QCW:
                              PAD + off + (ch + 1) * QCW],
                )
                nc.tensor.matmul(
                    pss[0:1, :QCW], ones[:, 0:1], prod[:, :],
                    start=True, stop=True,
                )
                s_t = sb.tile([1, QCW], f32, tag="st", bufs=1, name="s_t")
                nc.vector.tensor_copy(s_t[:, :], pss[0:1, :QCW])
                nc.gpsimd.dma_start(
                    sraw_b[j:j + 1, ch * QCW:(ch + 1) * QCW], s_t[:, :]
                )
        nc.gpsimd.collective_compute(
            "AllReduce",
            mybir.AluOpType.add,
            ins=[sraw_b.opt()],
            outs=[sred_b.opt()],
            replica_groups=[list(range(NC))],
        )

        # ========== A phase: A1 = W1a^T @ hu (chunk-major, overlaps AR) =====
        for chx in range(NMC):
            hu_c = hmc_tile(f"hu{chx}")
            nc.sync.dma_start(
                hu_c[:, :, :],
                huT.ap().rearrange("(o p) m -> p o m", p=128)[
                    :, :, chx * MCW:(chx + 1) * MCW],
            )
            for hg in range(0, HB, HGS):
                psa = [psum_mm(f"psa{gi}") for gi in range(HGS)]
                for dbb in range(0, DB, DSLAB):
                    w_t = w1t_tile("w1a_t")
                    nc.sync.dma_start(
                        w_t[:, :, :],
                        w1a[dbb * 128:(dbb + DSLAB) * 128,
                            hg * 128:(hg + HGS) * 128].rearrange(
                                "(o p) h -> p o h", p=128),
                    )
                    for i in range(DSLAB):
                        db = dbb + i
                        for gi in range(HGS):
                            nc.tensor.matmul(
                                psa[gi][:, :MCW],
                                w_t[:, i, gi * 128:(gi + 1) * 128],
                                hu_c[:, db, :],
                                start=db == 0, stop=db == DB - 1,
                            )
                for gi in range(HGS):
                    nc.scalar.activation(
                        A1T[:, hg + gi,
                            PAD + chx * MCW: PAD + (chx + 1) * MCW],
                        psa[gi][:, :MCW], AF.Identity,
                        bias=0.0, scale=1.0,
                    )

        # ========== phase 2: softmax / combine (overlaps A phase) ==========
        ncopies = max(1, NC // max(1, DPB))
        inv_sqrt_dp = 1.0 / (float(np.sqrt(DP)) * ncopies)
        sr2 = sb.tile([J, M], f32, name="sr2")
        nc.gpsimd.dma_start(sr2[:, :], sred_b[:, :])
        nc.scalar.activation(
            ej[:, :], sr2[:, :], AF.Exp, bias=0.0, scale=inv_sqrt_dp,
        )
        nc.vector.tensor_mul(out=ej[:, :], in0=ej[:, :], in1=vm_sb[:, :])
        # r0 = 1/max(sum_j ej, eps); g0 = 1/sum_k eg; g1 = eg[esel]
        for half in range(M // 512):
            hsl = slice(half * 512, (half + 1) * 512)
            p1 = psum_row("p_rsum")
            nc.tensor.matmul(p1[0:1, :], ones[0:J, 0:1], ej[:, hsl],
                             start=True, stop=True)
            nc.vector.tensor_copy(r0[0:1, hsl], p1[0:1, :])
            p2 = psum_row("p_gsum")
            nc.tensor.matmul(p2[0:1, :], ones[0:K, 0:1], eg_sb[:, hsl],
                             start=True, stop=True)
            nc.vector.tensor_copy(g0[0:1, hsl], p2[0:1, :])
            p3 = psum_row("p_gsel")
            nc.tensor.matmul(p3[0:1, :], esel_sb[:, 0:1], eg_sb[:, hsl],
                             start=True, stop=True)
            nc.vector.tensor_copy(g1[0:1, hsl], p3[0:1, :])
        if not b2z:
            nc.vector.tensor_copy(cwsum_bf[0:1, :], r0[0:1, :])  # rowsum
        nc.vector.tensor_scalar_max(r0[0:1, :], r0[0:1, :], 1e-8)
        with nc.allow_low_precision(reason="bf16 softmax denominators"):
            nc.vector.reciprocal(r0[0:1, :], r0[0:1, :])
            nc.vector.reciprocal(g0[0:1, :], g0[0:1, :])
        nc.vector.tensor_mul(out=g1[0:1, :], in0=g1[0:1, :], in1=g0[0:1, :])
        # w = gate[e] / rowsum, broadcast to [J, M]; fold into ej in place
        nc.vector.tensor_mul(out=g1[0:1, :], in0=g1[0:1, :], in1=r0[0:1, :])
        if not b2z:
            # cwsum = rowsum * w  (~gate[e], but 0 for pairless masks)
            nc.vector.tensor_mul(out=cwsum_bf[0:1, :], in0=cwsum_bf[0:1, :],
                                 in1=g1[0:1, :])
        for half in range(M // 512):
            hsl = slice(half * 512, (half + 1) * 512)
            pw = psum_row("p_wbj")
            nc.tensor.matmul(pw[0:J, :], ones[0:1, 0:J], g1[0:1, hsl],
                             start=True, stop=True)
            nc.vector.tensor_copy(wbJ[:, hsl], pw[0:J, :])
        nc.vector.tensor_mul(out=ej[:, :], in0=ej[:, :], in1=wbJ[:, :])

        # cwb: broadcast combine-weight rows across partitions
        cwb_store = {}
        for mc in range(NMC):
            cwb = cwb_tile(f"cwb{mc}")
            for j in range(J):
                psb = psum_mm(f"psb{j}")
                nc.tensor.matmul(
                    psb[:, :MCW], sel_bc[:, j * 128:(j + 1) * 128],
                    ej[:, mc * MCW:(mc + 1) * MCW],
                    start=True, stop=True,
                )
                nc.vector.tensor_copy(cwb[:, j, :], psb[:, :MCW])
            cwb_store[mc] = cwb

        # ================= steady pipeline: M -> hid -> W2 =================
        def m_phase(mc, mh_t):
            hm_c = hmc_tile(f"hm{mc}")
            nc.sync.dma_start(
                hm_c[:, :, :],
                hmT.ap().rearrange("(o p) m -> p o m", p=128)[
                    :, :, mc * MCW:(mc + 1) * MCW],
            )
            for hg in range(0, HB, HGS):
                psm = [psum_mm(f"psm{gi}") for gi in range(HGS)]
                for dbb in range(0, DB, DSLAB):
                    w_t = w1t_tile("w1b_t")
                    nc.sync.dma_start(
                        w_t[:, :, :],
                        w1b[dbb * 128:(dbb + DSLAB) * 128,
                            hg * 128:(hg + HGS) * 128].rearrange(
                                "(o p) h -> p o h", p=128),
                    )
                    for i in range(DSLAB):
                        db = dbb + i
                        for gi in range(HGS):
                            nc.tensor.matmul(
                                psm[gi][:, :MCW],
                                w_t[:, i, gi * 128:(gi + 1) * 128],
                                hm_c[:, db, :],
                                start=db == 0, stop=db == DB - 1,
                            )
                for gi in range(HGS):
                    hb = hg + gi
                    nc.scalar.activation(
                        mh_t[:, hb, :], psm[gi][:, :MCW], AF.Identity,
                        bias=b1_sb[:, hb:hb + 1], scale=1.0,
                    )

        def hid_phase(mc, mh_t, cwb):
            c0 = mc * MCW
            for hb in range(HB):
                scr = sb.tile([128, MCW], bf16, tag="scr", bufs=2, name="scr")
                for j in range(J):
                    off = int(offs[j])
                    x_t = sb.tile([128, MCW], bf16, tag="xt", bufs=2,
                                  name="x_t")
                    nc.vector.tensor_add(
                        out=x_t[:, :],
                        in0=A1T[:, hb, PAD + off + c0: PAD + off + c0 + MCW],
                        in1=mh_t[:, hb, :],
                    )
                    g_t = sb.tile([128, MCW], bf16, tag="gt", bufs=2,
                                  name="g_t")
                    nc.scalar.activation(
                        g_t[:, :], x_t[:, :], hid_af, bias=0.0, scale=1.0,
                    )
                    if j == 0:
                        dst = scr[:, :] if J > 1 else mh_t[:, hb, :]
                        nc.vector.tensor_mul(
                            out=dst, in0=g_t[:, :], in1=cwb[:, j, :],
                        )
                    elif j < J - 1:
                        nc.vector.tensor_mul(
                            out=g_t[:, :], in0=g_t[:, :], in1=cwb[:, j, :]
                        )
                        nc.vector.tensor_add(
                            out=scr[:, :], in0=scr[:, :], in1=g_t[:, :]
                        )
                    else:
                        nc.vector.tensor_mul(
                            out=g_t[:, :], in0=g_t[:, :], in1=cwb[:, j, :]
                        )
                        nc.vector.tensor_add(
                            out=mh_t[:, hb, :], in0=scr[:, :], in1=g_t[:, :]
                        )

        def w2_phase(mc, mh_t):
            c0 = mc * MCW
            for dg in range(0, DB, DGS):
                psd = [psum_mm(f"psd{gi}") for gi in range(DGS)]
                for hbb in range(0, HB, HSLAB):
                    w2_t = w2t_tile("w2_t")
                    nc.sync.dma_start(
                        w2_t[:, :, :],
                        w2[hbb * 128:(hbb + HSLAB) * 128,
                           dg * 128:(dg + DGS) * 128].rearrange(
                               "(o p) d -> p o d", p=128),
                    )
                    for i in range(HSLAB):
                        hb = hbb + i
                        for gi in range(DGS):
                            nc.tensor.matmul(
                                psd[gi][:, :MCW],
                                w2_t[:, i, gi * 128:(gi + 1) * 128],
                                mh_t[:, hb, :],
                                start=hb == 0,
                                stop=(hb == HB - 1) if b2z else False,
                            )
                for gi in range(DGS):
                    db = dg + gi
                    if not b2z:
                        nc.tensor.matmul(
                            psd[gi][:, :MCW],
                            b2_sb[0:1, db * 128:(db + 1) * 128],
                            cwsum_bf[0:1, c0:c0 + MCW],
                            start=False, stop=True,
                        )
                    d_t = sb.tile([128, MCW], bf16, tag="dt", bufs=2,
                                  name="d_t")
                    nc.scalar.activation(
                        d_t[:, :], psd[gi][:, :MCW], AF.Identity,
                        bias=0.0, scale=1.0,
                    )
                    for hx in range(MCW // RSW):
                        g = (c0 // RSW) + hx
                        nc.sync.dma_start(
                            bounce[g][db * 128:(db + 1) * 128, :],
                            d_t[:, hx * RSW:(hx + 1) * RSW],
                        )

        def rs_phase(mc):
            for hx in range(MCW // RSW):
                g = (mc * MCW) // RSW + hx
                nc.gpsimd.collective_compute(
                    "ReduceScatter",
                    mybir.AluOpType.add,
                    ins=[bounce[g].opt()],
                    outs=[rsout[g].opt()],
                    replica_groups=[list(range(NC))],
                )
                nc.gpsimd.dma_start(
                    outp[:, g * RSW:(g + 1) * RSW], rsout[g][:, :]
                )

        mh_store = {}
        for mc in range(NMC):
            mh_store[mc] = mh_tile(f"mh{mc}")
            m_phase(mc, mh_store[mc])
            hid_phase(mc, mh_store[mc], cwb_store[mc])
            if mc >= 1:
                w2_phase(mc - 1, mh_store[mc - 1])
                rs_phase(mc - 1)
                del mh_store[mc - 1]
        mc = NMC - 1
        w2_phase(mc, mh_store[mc])
        rs_phase(mc)

    nc.finalize()
    return nc


def _prepare(inputs, cfg):
    import ml_dtypes
    BF16 = ml_dtypes.bfloat16
    D, H, M, U, DP, K = cfg["D"], cfg["H"], cfg["M"], cfg["U"], cfg["DP"], cfg["K"]
    HB, DPB = H // 128, DP // 128
    offs, valid = cfg["offs"], cfg["valid"]
    J = len(offs)

    h = np.asarray(inputs["h_L"], dtype=np.float32)[0]
    m_idx = np.asarray(inputs["mask_indices"]).astype(np.int64)
    u_idx = np.asarray(inputs["unmasked_indices"]).astype(np.int64)

    hmT = np.ascontiguousarray(h[m_idx].astype(BF16).T)
    huT = np.ascontiguousarray(h[u_idx].astype(BF16).T)
    wq = np.asarray(inputs["Wq"], np.float32).astype(BF16)
    wk = np.asarray(inputs["Wk"], np.float32).astype(BF16)
    wr = np.asarray(inputs["Wr"], np.float32).astype(BF16)
    bq = np.asarray(inputs["bq"], np.float32)
    bk = np.asarray(inputs["bk"], np.float32)
    brc = np.zeros((128, 1), np.float32)
    brc[:K, 0] = np.asarray(inputs["br"], np.float32)
    vm = np.ascontiguousarray(valid).astype(BF16)  # [J, M]

    W1 = np.asarray(inputs["W1"], np.float32)
    W2 = np.asarray(inputs["W2"], np.float32)
    b1 = np.asarray(inputs["b1"], np.float32)
    b2 = np.asarray(inputs["b2"], np.float32)

    selbc_h = np.zeros((J, J * 128), dtype=BF16)
    for j in range(J):
        selbc_h[j, j * 128:(j + 1) * 128] = 1.0

    DPBT = max(1, DP // 128)
    in_maps = []
    for c in range(cfg["NC"]):
        e = c % K
        dpb = c % DPBT
        dsl = slice(dpb * 128, (dpb + 1) * 128)
        sel = np.zeros((K, 1), np.float32)
        sel[e, 0] = 1.0
        in_maps.append({
            "hmT": hmT, "huT": huT,
            "w1a": np.ascontiguousarray(W1[e][:D]).astype(BF16),
            "w1b": np.ascontiguousarray(W1[e][D:]).astype(BF16),
            "w2": W2[e].astype(BF16),
            "wq": np.ascontiguousarray(wq[:, dsl]),
            "wk": np.ascontiguousarray(wk[:, dsl]),
            "wr": wr,
            "b1c": np.ascontiguousarray(b1[e].reshape(HB, 128).T),
            "b2r": b2[e].reshape(1, D).astype(BF16),
            "bqc": np.ascontiguousarray(bq[dsl].reshape(128, 1)),
            "bkc": np.ascontiguousarray(bk[dsl].reshape(128, 1)),
            "brc": brc,
            "esel": sel.astype(BF16), "vmask": vm, "selbc": selbc_h,
        })
    return in_maps, m_idx


def _run(cfg, in_maps, trace=False, sim=False):
    global LAST_RESULT
    key = cfg["key"]
    if key not in _GRAPH_CACHE:
        _GRAPH_CACHE[key] = build_graph(cfg)
    nc = _GRAPH_CACHE[key]
    if sim:
        from concourse import bass_interp
        s = bass_interp.MultiCoreSim(nc, cfg["NC"])
        for c in range(cfg["NC"]):
            for k, v in in_maps[c].items():
                s.cores[c].tensor(k)[:] = v
        s.simulate(check_with_hw=False)
        return [{"out": np.asarray(s.cores[c].mem_tensor("out"))}
                for c in range(cfg["NC"])]
    from concourse import bass_utils
    kw = {}
    if trace and os.environ.get("KERNEL_TRACE_DIR"):
        kw["tmpdir"] = os.environ["KERNEL_TRACE_DIR"]
    res = bass_utils.run_bass_kernel_spmd(
        nc, in_maps, core_ids=list(range(cfg["NC"])), trace=trace, **kw,
    )
    LAST_RESULT = res
    return res.results


def kernel_impl(inputs, D, K, L, M, U, DP, H, NC, MCW, QCW, NRS, sim=False,
                hid_act="Gelu", SCW=None):
    PMAX = M * 10

    m_idx = np.asarray(inputs["mask_indices"]).astype(np.int64)
    u_idx = np.asarray(inputs["unmasked_indices"]).astype(np.int64)
    r = int(np.asarray(inputs["range_r"]))

    offs, valid = build_tables(m_idx, u_idx, r, PMAX)
    J = len(offs)
    if J == 0:
        return np.zeros((1, L, D), np.float32)
    PAD = int(max(8, np.max(np.abs(offs))))
    PAD = (PAD + 7) // 8 * 8
    b2z = not np.any(np.asarray(inputs["b2"]))

    cfg = {
        "D": D, "H": H, "M": M, "U": U, "DP": DP, "K": K, "NC": NC,
        "offs": offs, "valid": valid, "PAD": PAD, "b2z": b2z,
        "MCW": MCW, "QCW": QCW, "NRS": NRS, "hid_act": hid_act,
        "key": (D, H, M, U, DP, K, NC, MCW, QCW, NRS, PAD, hid_act, b2z,
                tuple(offs.tolist())),
    }

    in_maps, m_idx = _prepare(inputs, cfg)
    results = _run(cfg, in_maps, trace=bool(os.environ.get("KERNEL_TRACE")),
                   sim=sim)

    deltaT = np.concatenate(
        [np.asarray(results[c]["out"], np.float32) for c in range(NC)], axis=0
    )  # [D, M]
    delta_md = deltaT.T  # [M, D]
    out = np.zeros((L, D), np.float32)
    if len(np.unique(m_idx)) == len(m_idx):
        out[m_idx] = delta_md
    else:
        np.add.at(out, m_idx, delta_md)
    return out[None]


def kernel(**inputs):
    return kernel_impl(
        inputs, D=4096, K=8, L=2048, M=1024, U=1024, DP=512, H=2048,
        NC=NCORES, MCW=512, QCW=512, NRS=4,
    )


# revision 22
# speedup vs baseline: 1.0773x; 1.0773x over previous
"""Trainium2 Bass kernel for nn_AMIPRouterInference (gnn_message_passing).

v2: flat pool (no phase barriers), A-phase overlapped with score AllReduce +
softmax, in-place m1b->hid buffer merge, FD=512 vector ops, partition-parallel
softmax, SBUF-resident combine weights, NRS=4 reduce-scatter.
"""

import os
import numpy as np

NCORES = 8

_GRAPH_CACHE = {}
LAST_RESULT = None  # BassKernelResults of the most recent device run


def build_tables(m_idx, u_idx, r, pmax):
    M = len(m_idx)
    dists = np.abs(m_idx[:, None].astype(np.int64) - u_idx[None, :].astype(np.int64))
    adj = (dists > 0) & (dists <= r)
    pair_m, pair_u = np.nonzero(adj)  # row-major == jnp.nonzero order
    pair_m = pair_m[:pmax]
    pair_u = pair_u[:pmax]
    offs = np.unique(pair_u - pair_m).astype(np.int64)
    J = len(offs)
    valid = np.zeros((J, M), dtype=np.float32)
    for j, d in enumerate(offs):
        valid[j, pair_m[(pair_u - pair_m) == d]] = 1.0
    return offs, valid


def build_graph(cfg):
    import contextlib
    import concourse.mybir as mybir
    import concourse.tile as tile
    from concourse import bacc

    D, H, M, U, DP, K = cfg["D"], cfg["H"], cfg["M"], cfg["U"], cfg["DP"], cfg["K"]
    NC = cfg["NC"]
    offs = cfg["offs"]
    J = len(offs)
    PAD = cfg["PAD"]
    MCW = cfg["MCW"]            # compute chunk width along M
    NMC = M // MCW
    QCW = cfg["QCW"]            # qk/score-phase chunk width
    NQC = M // QCW
    DB, HB, DPB = D // 128, H // 128, DP // 128
    HGS = 2                     # h-blocks per A/M-phase psum group
    DGS = 4                     # d-blocks per W2-phase psum group
    DSLAB = 4                   # d-blocks per weight DMA slab
    HSLAB = 4                   # h-blocks per W2 weight DMA slab
    RSD = D // NC               # rows of final output per core
    NRS = cfg["NRS"]            # number of reduce-scatter column groups
    RSW = M // NRS
    b2z = cfg.get("b2z", False)
    assert M % MCW == 0 and M % QCW == 0 and M % NRS == 0 and MCW % RSW == 0

    bf16 = mybir.dt.bfloat16
    f32 = mybir.dt.float32
    AF = mybir.ActivationFunctionType
    hid_af = getattr(AF, cfg.get("hid_act", "Gelu"))

    nc = bacc.Bacc(None, target_bir_lowering=False, debug=False)

    # ---------------- DRAM parameters ----------------
    hmT = nc.declare_dram_parameter("hmT", [D, M], bf16, isOutput=False)
    huT = nc.declare_dram_parameter("huT", [D, U], bf16, isOutput=False)
    w1a = nc.declare_dram_parameter("w1a", [D, H], bf16, isOutput=False)
    w1b = nc.declare_dram_parameter("w1b", [D, H], bf16, isOutput=False)
    w2 = nc.declare_dram_parameter("w2", [H, D], bf16, isOutput=False)
    wq = nc.declare_dram_parameter("wq", [D, 128], bf16, isOutput=False)
    wk = nc.declare_dram_parameter("wk", [D, 128], bf16, isOutput=False)
    wr = nc.declare_dram_parameter("wr", [D, K], bf16, isOutput=False)
    b1c = nc.declare_dram_parameter("b1c", [128, HB], f32, isOutput=False)
    b2r = nc.declare_dram_parameter("b2r", [1, D], bf16, isOutput=False)
    bqc = nc.declare_dram_parameter("bqc", [128, 1], f32, isOutput=False)
    bkc = nc.declare_dram_parameter("bkc", [128, 1], f32, isOutput=False)
    brc = nc.declare_dram_parameter("brc", [128, 1], f32, isOutput=False)
    esel = nc.declare_dram_parameter("esel", [K, 1], bf16, isOutput=False)
    selbc = nc.declare_dram_parameter("selbc", [J, J * 128], bf16,
                                      isOutput=False)
    vmask = nc.declare_dram_parameter("vmask", [J, M], bf16, isOutput=False)
    outp = nc.declare_dram_parameter("out", [RSD, M], bf16, isOutput=True)

    with tile.TileContext(nc) as tc, contextlib.ExitStack() as ctx:
        sb = ctx.enter_context(tc.tile_pool(name="sb", bufs=1))
        ps = ctx.enter_context(tc.tile_pool(name="ps", bufs=1, space="PSUM"))
        dram = ctx.enter_context(tc.tile_pool(name="dram", bufs=1, space="DRAM"))

        def psum_mm(name):
            return ps.tile([128, 512], f32, tag="mm", bufs=6, name=name)

        def psum_row(name):
            return ps.tile([16, 512], f32, tag="row", bufs=2, name=name)

        # ---------------- persistent SBUF tensors ----------------
        ones = sb.tile([128, 128], bf16, name="ones")
        nc.vector.memset(ones[:, :], 1.0)
        ones32 = sb.tile([128, 1], f32, name="ones32")
        nc.vector.memset(ones32[:, :], 1.0)

        b1_sb = sb.tile([128, HB], f32, name="b1_sb")
        nc.sync.dma_start(b1_sb[:, :], b1c[:, :])
        bq_sb = sb.tile([128, 1], f32, name="bq_sb")
        nc.sync.dma_start(bq_sb[:, :], bqc[:, :])
        bk_sb = sb.tile([128, 1], f32, name="bk_sb")
        nc.sync.dma_start(bk_sb[:, :], bkc[:, :])
        br_sb = sb.tile([128, 1], f32, name="br_sb")
        nc.sync.dma_start(br_sb[:, :], brc[:, :])
        esel_sb = sb.tile([K, 1], bf16, name="esel_sb")
        nc.sync.dma_start(esel_sb[:, :], esel[:, :])
        wr_sb = sb.tile([128, DB, K], bf16, name="wr_sb")
        nc.sync.dma_start(
            wr_sb[:, :, :], wr.ap().rearrange("(o p) k -> p o k", p=128)
        )
        if not b2z:
            b2_sb = sb.tile([1, D], bf16, name="b2_sb")
            nc.sync.dma_start(b2_sb[:, :], b2r[:, :])
        vm_sb = sb.tile([J, M], bf16, name="vm_sb")
        nc.sync.dma_start(vm_sb[:, :], vmask[:, :])

        # big persistent tensors
        A1T = sb.tile([128, HB, U + 2 * PAD], bf16, name="A1T")
        for hb in range(HB):
            nc.vector.memset(A1T[:, hb, 0:PAD], 0.0)
            nc.vector.memset(A1T[:, hb, PAD + U: U + 2 * PAD], 0.0)

        def mh_tile(name):
            # holds M1 (post-bias) per chunk, overwritten in place by hid
            return sb.tile([128, HB, MCW], bf16, tag="mh", bufs=2, name=name)

        def hmc_tile(name):
            # streamed h chunks (hu for A phase, hm for M phase)
            return sb.tile([128, DB, MCW], bf16, tag="hmc", bufs=2, name=name)

        def w1t_tile(name):
            return sb.tile([128, DSLAB, HGS * 128], bf16, tag="w1t", bufs=3,
                           name=name)

        def w2t_tile(name):
            return sb.tile([128, HSLAB, DGS * 128], bf16, tag="w2t", bufs=3,
                           name=name)

        def cwb_tile(name):
            return sb.tile([128, J, MCW], bf16, tag="cwb", bufs=1, name=name)

        # qk / softmax phase tiles
        kT_sb = sb.tile([128, U + 2 * PAD], bf16, name="kT_sb")
        nc.vector.memset(kT_sb[:, 0:PAD], 0.0)
        nc.vector.memset(kT_sb[:, PAD + U: U + 2 * PAD], 0.0)
        qT_sb = sb.tile([128, M], bf16, name="qT_sb")
        eg_sb = sb.tile([K, M], bf16, name="eg_sb")
        ej = sb.tile([J, M], bf16, name="ej")
        r0 = sb.tile([1, M], bf16, name="r0")
        g0 = sb.tile([1, M], bf16, name="g0")
        g1 = sb.tile([1, M], bf16, name="g1")
        wbJ = sb.tile([J, M], bf16, name="wbJ")
        cwsum_bf = sb.tile([1, M], bf16, name="cwsum_bf") if not b2z else None
        # one-hot selectors: sel_bc[c, j*128+p] = (c == j), used as matmul lhsT
        # to broadcast row j of a [J, M] tile across 128 partitions
        sel_bc = sb.tile([J, J * 128], bf16, name="sel_bc")
        nc.sync.dma_start(sel_bc[:, :], selbc[:, :])

        sraw_b = dram.tile([J, M], f32, name="sraw_b")
        sred_b = dram.tile(
            [J, M], f32, name="sred_b",
            addr_space="Shared" if NC > 4 else "Local",
        )
        bounce = [
            dram.tile([D, RSW], bf16, name=f"bounce{g}") for g in range(NRS)
        ]
        rsout = [
            dram.tile([RSD, RSW], bf16, name=f"rsout{g}") for g in range(NRS)
        ]

        # ================= phase 1: q/k/gate + raw scores =================
        DBB = 2

        def qk_mm(ch):
            csl = slice(ch * QCW, (ch + 1) * QCW)
            psq = psum_mm("psq")
            psg = psum_row("psg")
            for dbb in range(0, DB, DBB):
                hm_t = sb.tile([128, DBB, QCW], bf16, tag="ht", bufs=2,
                               name="hm_t")
                nc.sync.dma_start(
                    hm_t[:, :, :],
                    hmT[dbb * 128:(dbb + DBB) * 128, csl].rearrange(
                        "(o p) m -> p o m", p=128),
                )
                wq_t = sb.tile([128, DBB, 128], bf16, tag="wt", bufs=2,
                               name="wq_t")
                nc.sync.dma_start(
                    wq_t[:, :, :],
                    wq[dbb * 128:(dbb + DBB) * 128, :].rearrange(
                        "(o p) m -> p o m", p=128),
                )
                for i in range(DBB):
                    db = dbb + i
                    st, sp = db == 0, db == DB - 1
                    nc.tensor.matmul(
                        psq[:, :QCW], wq_t[:, i, :], hm_t[:, i, :],
                        start=st, stop=sp,
                    )
                    nc.tensor.matmul(
                        psg[:K, :QCW], wr_sb[:, db, :], hm_t[:, i, :],
                        start=st, stop=sp,
                    )
            nc.vector.tensor_scalar_add(
                qT_sb[:, csl], psq[:, :QCW], bq_sb[:, 0:1],
            )
            nc.scalar.activation(
                eg_sb[:, csl], psg[:K, :QCW], AF.Exp,
                bias=br_sb[0:K, 0:1], scale=1.0,
            )
            psk = psum_mm("psk")
            for dbb in range(0, DB, DBB):
                hu_t = sb.tile([128, DBB, QCW], bf16, tag="ht", bufs=2,
                               name="hu_t")
                nc.sync.dma_start(
                    hu_t[:, :, :],
                    huT[dbb * 128:(dbb + DBB) * 128, csl].rearrange(
                        "(o p) m -> p o m", p=128),
                )
                wk_t = sb.tile([128, DBB, 128], bf16, tag="wt", bufs=2,
                               name="wk_t")
                nc.sync.dma_start(
                    wk_t[:, :, :],
                    wk[dbb * 128:(dbb + DBB) * 128, :].rearrange(
                        "(o p) m -> p o m", p=128),
                )
                for i in range(DBB):
                    db = dbb + i
                    nc.tensor.matmul(
                        psk[:, :QCW], wk_t[:, i, :], hu_t[:, i, :],
                        start=db == 0, stop=db == DB - 1,
                    )
            nc.vector.tensor_scalar_add(
                kT_sb[:, PAD + ch * QCW: PAD + (ch + 1) * QCW],
                psk[:, :QCW], bk_sb[:, 0:1],
            )

        def scores_all():
            for ch in range(NQC):
                qk_scores(ch)
            nc.gpsimd.collective_compute(
                "AllReduce",
                mybir.AluOpType.add,
                ins=[sraw_b.opt()],
                outs=[sred_b.opt()],
                replica_groups=[list(range(NC))],
            )

        def qk_scores(ch):
            for j in range(J):
                off = int(offs[j])
                pss = psum_row(f"pss{j}")
                prod = sb.tile([128, QCW], bf16, tag="prod", bufs=2,
                               name="prod")
                nc.vector.tensor_mul(
                    out=prod[:, :],
                    in0=qT_sb[:, ch * QCW:(ch + 1) * QCW],
                    in1=kT_sb[:, PAD + off + ch * QCW:
                              PAD + off + (ch + 1) * QCW],
                )
                nc.tensor.matmul(
                    pss[0:1, :QCW], ones[:, 0:1], prod[:, :],
                    start=True, stop=True,
                )
                s_t = sb.tile([1, QCW], f32, tag="st", bufs=1, name="s_t")
                nc.vector.tensor_copy(s_t[:, :], pss[0:1, :QCW])
                nc.gpsimd.dma_start(
                    sraw_b[j:j + 1, ch * QCW:(ch + 1) * QCW], s_t[:, :]
                )
        # ========== A phase: A1 = W1a^T @ hu (chunk-major, overlaps AR) =====
        def a_chunk(chx):
            hu_c = hmc_tile(f"hu{chx}")
            nc.sync.dma_start(
                hu_c[:, :, :],
                huT.ap().rearrange("(o p) m -> p o m", p=128)[
                    :, :, chx * MCW:(chx + 1) * MCW],
            )
            for hg in range(0, HB, HGS):
                psa = [psum_mm(f"psa{gi}") for gi in range(HGS)]
                for dbb in range(0, DB, DSLAB):
                    w_t = w1t_tile("w1a_t")
                    nc.sync.dma_start(
                        w_t[:, :, :],
                        w1a[dbb * 128:(dbb + DSLAB) * 128,
                            hg * 128:(hg + HGS) * 128].rearrange(
                                "(o p) h -> p o h", p=128),
                    )
                    for i in range(DSLAB):
                        db = dbb + i
                        for gi in range(HGS):
                            nc.tensor.matmul(
                                psa[gi][:, :MCW],
                                w_t[:, i, gi * 128:(gi + 1) * 128],
                                hu_c[:, db, :],
                                start=db == 0, stop=db == DB - 1,
                            )
                for gi in range(HGS):
                    nc.scalar.activation(
                        A1T[:, hg + gi,
                            PAD + chx * MCW: PAD + (chx + 1) * MCW],
                        psa[gi][:, :MCW], AF.Identity,
                        bias=0.0, scale=1.0,
                    )

        # interleave: qk(0), A(0), qk(1), scores+AllReduce, A(1)
        qk_mm(0)
        a_chunk(0)
        for ch in range(1, NQC):
            qk_mm(ch)
        scores_all()
        for chx in range(1, NMC):
            a_chunk(chx)

        # ========== phase 2: softmax / combine (overlaps A phase) ==========
        ncopies = max(1, NC // max(1, DPB))
        inv_sqrt_dp = 1.0 / (float(np.sqrt(DP)) * ncopies)
        sr2 = sb.tile([J, M], f32, name="sr2")
        nc.gpsimd.dma_start(sr2[:, :], sred_b[:, :])
        nc.scalar.activation(
            ej[:, :], sr2[:, :], AF.Exp, bias=0.0, scale=inv_sqrt_dp,
        )
        nc.vector.tensor_mul(out=ej[:, :], in0=ej[:, :], in1=vm_sb[:, :])
        # r0 = 1/max(sum_j ej, eps); g0 = 1/sum_k eg; g1 = eg[esel]
        for half in range(M // 512):
            hsl = slice(half * 512, (half + 1) * 512)
            p1 = psum_row("p_rsum")
            nc.tensor.matmul(p1[0:1, :], ones[0:J, 0:1], ej[:, hsl],
                             start=True, stop=True)
            nc.vector.tensor_copy(r0[0:1, hsl], p1[0:1, :])
            p2 = psum_row("p_gsum")
            nc.tensor.matmul(p2[0:1, :], ones[0:K, 0:1], eg_sb[:, hsl],
                             start=True, stop=True)
            nc.vector.tensor_copy(g0[0:1, hsl], p2[0:1, :])
            p3 = psum_row("p_gsel")
            nc.tensor.matmul(p3[0:1, :], esel_sb[:, 0:1], eg_sb[:, hsl],
                             start=True, stop=True)
            nc.vector.tensor_copy(g1[0:1, hsl], p3[0:1, :])
        if not b2z:
            nc.vector.tensor_copy(cwsum_bf[0:1, :], r0[0:1, :])  # rowsum
        nc.vector.tensor_scalar_max(r0[0:1, :], r0[0:1, :], 1e-8)
        with nc.allow_low_precision(reason="bf16 softmax denominators"):
            nc.vector.reciprocal(r0[0:1, :], r0[0:1, :])
            nc.vector.reciprocal(g0[0:1, :], g0[0:1, :])
        nc.vector.tensor_mul(out=g1[0:1, :], in0=g1[0:1, :], in1=g0[0:1, :])
        # w = gate[e] / rowsum, broadcast to [J, M]; fold into ej in place
        nc.vector.tensor_mul(out=g1[0:1, :], in0=g1[0:1, :], in1=r0[0:1, :])
        if not b2z:
            # cwsum = rowsum * w  (~gate[e], but 0 for pairless masks)
            nc.vector.tensor_mul(out=cwsum_bf[0:1, :], in0=cwsum_bf[0:1, :],
                                 in1=g1[0:1, :])
        for half in range(M // 512):
            hsl = slice(half * 512, (half + 1) * 512)
            pw = psum_row("p_wbj")
            nc.tensor.matmul(pw[0:J, :], ones[0:1, 0:J], g1[0:1, hsl],
                             start=True, stop=True)
            nc.vector.tensor_copy(wbJ[:, hsl], pw[0:J, :])
        nc.vector.tensor_mul(out=ej[:, :], in0=ej[:, :], in1=wbJ[:, :])

        # cwb: broadcast combine-weight rows across partitions (built lazily
        # per chunk so the single buffer slot never inverts engine order)
        def cwb_build(mc):
            cwb = cwb_tile(f"cwb{mc}")
            for j in range(J):
                psb = psum_mm(f"psb{j}")
                nc.tensor.matmul(
                    psb[:, :MCW], sel_bc[:, j * 128:(j + 1) * 128],
                    ej[:, mc * MCW:(mc + 1) * MCW],
                    start=True, stop=True,
                )
                nc.vector.tensor_copy(cwb[:, j, :], psb[:, :MCW])
            return cwb

        # ================= steady pipeline: M -> hid -> W2 =================
        def m_phase(mc, mh_t):
            hm_c = hmc_tile(f"hm{mc}")
            nc.sync.dma_start(
                hm_c[:, :, :],
                hmT.ap().rearrange("(o p) m -> p o m", p=128)[
                    :, :, mc * MCW:(mc + 1) * MCW],
            )
            for hg in range(0, HB, HGS):
                psm = [psum_mm(f"psm{gi}") for gi in range(HGS)]
                for dbb in range(0, DB, DSLAB):
                    w_t = w1t_tile("w1b_t")
                    nc.sync.dma_start(
                        w_t[:, :, :],
                        w1b[dbb * 128:(dbb + DSLAB) * 128,
                            hg * 128:(hg + HGS) * 128].rearrange(
                                "(o p) h -> p o h", p=128),
                    )
                    for i in range(DSLAB):
                        db = dbb + i
                        for gi in range(HGS):
                            nc.tensor.matmul(
                                psm[gi][:, :MCW],
                                w_t[:, i, gi * 128:(gi + 1) * 128],
                                hm_c[:, db, :],
                                start=db == 0, stop=db == DB - 1,
                            )
                for gi in range(HGS):
                    hb = hg + gi
                    nc.scalar.activation(
                        mh_t[:, hb, :], psm[gi][:, :MCW], AF.Identity,
                        bias=b1_sb[:, hb:hb + 1], scale=1.0,
                    )

        def hid_phase(mc, mh_t, cwb):
            c0 = mc * MCW
            for hb in range(HB):
                scr = sb.tile([128, MCW], bf16, tag="scr", bufs=2, name="scr")
                for j in range(J):
                    off = int(offs[j])
                    x_t = sb.tile([128, MCW], bf16, tag="xt", bufs=2,
                                  name="x_t")
                    nc.vector.tensor_add(
                        out=x_t[:, :],
                        in0=A1T[:, hb, PAD + off + c0: PAD + off + c0 + MCW],
                        in1=mh_t[:, hb, :],
                    )
                    g_t = sb.tile([128, MCW], bf16, tag="gt", bufs=2,
                                  name="g_t")
                    nc.scalar.activation(
                        g_t[:, :], x_t[:, :], hid_af, bias=0.0, scale=1.0,
                    )
                    if j == 0:
                        dst = scr[:, :] if J > 1 else mh_t[:, hb, :]
                        nc.vector.tensor_mul(
                            out=dst, in0=g_t[:, :], in1=cwb[:, j, :],
                        )
                    elif j < J - 1:
                        nc.vector.tensor_mul(
                            out=g_t[:, :], in0=g_t[:, :], in1=cwb[:, j, :]
                        )
                        nc.vector.tensor_add(
                            out=scr[:, :], in0=scr[:, :], in1=g_t[:, :]
                        )
                    else:
                        nc.vector.tensor_mul(
                            out=g_t[:, :], in0=g_t[:, :], in1=cwb[:, j, :]
                        )
                        nc.vector.tensor_add(
                            out=mh_t[:, hb, :], in0=scr[:, :], in1=g_t[:, :]
                        )

        def w2_pass(mc, mh_t, c0, cw):
            lo = c0 - mc * MCW
            for dg in range(0, DB, DGS):
                psd = [psum_mm(f"psd{gi}") for gi in range(DGS)]
                for hbb in range(0, HB, HSLAB):
                    w2_t = w2t_tile("w2_t")
                    nc.sync.dma_start(
                        w2_t[:, :, :],
                        w2[hbb * 128:(hbb + HSLAB) * 128,
                           dg * 128:(dg + DGS) * 128].rearrange(
                               "(o p) d -> p o d", p=128),
                    )
                    for i in range(HSLAB):
                        hb = hbb + i
                        for gi in range(DGS):
                            nc.tensor.matmul(
                                psd[gi][:, :cw],
                                w2_t[:, i, gi * 128:(gi + 1) * 128],
                                mh_t[:, hb, lo:lo + cw],
                                start=hb == 0,
                                stop=(hb == HB - 1) if b2z else False,
                            )
                for gi in range(DGS):
                    db = dg + gi
                    if not b2z:
                        nc.tensor.matmul(
                            psd[gi][:, :cw],
                            b2_sb[0:1, db * 128:(db + 1) * 128],
                            cwsum_bf[0:1, c0:c0 + cw],
                            start=False, stop=True,
                        )
                    d_t = sb.tile([128, MCW], bf16, tag="dt", bufs=2,
                                  name="d_t")
                    nc.scalar.activation(
                        d_t[:, :cw], psd[gi][:, :cw], AF.Identity,
                        bias=0.0, scale=1.0,
                    )
                    for hx in range(cw // RSW):
                        g = (c0 // RSW) + hx
                        nc.sync.dma_start(
                            bounce[g][db * 128:(db + 1) * 128, :],
                            d_t[:, hx * RSW:(hx + 1) * RSW],
                        )

        def rs_group(g):
            nc.gpsimd.collective_compute(
                "ReduceScatter",
                mybir.AluOpType.add,
                ins=[bounce[g].opt()],
                outs=[rsout[g].opt()],
                replica_groups=[list(range(NC))],
            )
            nc.gpsimd.dma_start(
                outp[:, g * RSW:(g + 1) * RSW], rsout[g][:, :]
            )

        mh_store = {}
        for mc in range(NMC):
            cwb_mc = cwb_build(mc)
            mh_store[mc] = mh_tile(f"mh{mc}")
            m_phase(mc, mh_store[mc])
            hid_phase(mc, mh_store[mc], cwb_mc)
            if mc >= 1:
                pm = mc - 1
                w2_pass(pm, mh_store[pm], pm * MCW, MCW)
                for hx in range(MCW // RSW):
                    rs_group(pm * MCW // RSW + hx)
                del mh_store[pm]
        mc = NMC - 1
        # split the last chunk's W2 by reduce-scatter group so the final
        # collectives overlap with compute instead of sitting on the tail
        for hx in range(MCW // RSW):
            c0 = mc * MCW + hx * RSW
            w2_pass(mc, mh_store[mc], c0, RSW)
            rs_group(c0 // RSW)

    nc.finalize()
    return nc


def _prepare(inputs, cfg):
    import ml_dtypes
    BF16 = ml_dtypes.bfloat16
    D, H, M, U, DP, K = cfg["D"], cfg["H"], cfg["M"], cfg["U"], cfg["DP"], cfg["K"]
    HB, DPB = H // 128, DP // 128
    offs, valid = cfg["offs"], cfg["valid"]
    J = len(offs)

    h = np.asarray(inputs["h_L"], dtype=np.float32)[0]
    m_idx = np.asarray(inputs["mask_indices"]).astype(np.int64)
    u_idx = np.asarray(inputs["unmasked_indices"]).astype(np.int64)

    hmT = np.ascontiguousarray(h[m_idx].astype(BF16).T)
    huT = np.ascontiguousarray(h[u_idx].astype(BF16).T)
    wq = np.asarray(inputs["Wq"], np.float32).astype(BF16)
    wk = np.asarray(inputs["Wk"], np.float32).astype(BF16)
    wr = np.asarray(inputs["Wr"], np.float32).astype(BF16)
    bq = np.asarray(inputs["bq"], np.float32)
    bk = np.asarray(inputs["bk"], np.float32)
    brc = np.zeros((128, 1), np.float32)
    brc[:K, 0] = np.asarray(inputs["br"], np.float32)
    vm = np.ascontiguousarray(valid).astype(BF16)  # [J, M]

    W1 = np.asarray(inputs["W1"], np.float32)
    W2 = np.asarray(inputs["W2"], np.float32)
    b1 = np.asarray(inputs["b1"], np.float32)
    b2 = np.asarray(inputs["b2"], np.float32)

    selbc_h = np.zeros((J, J * 128), dtype=BF16)
    for j in range(J):
        selbc_h[j, j * 128:(j + 1) * 128] = 1.0

    DPBT = max(1, DP // 128)
    in_maps = []
    for c in range(cfg["NC"]):
        e = c % K
        dpb = c % DPBT
        dsl = slice(dpb * 128, (dpb + 1) * 128)
        sel = np.zeros((K, 1), np.float32)
        sel[e, 0] = 1.0
        in_maps.append({
            "hmT": hmT, "huT": huT,
            "w1a": np.ascontiguousarray(W1[e][:D]).astype(BF16),
            "w1b": np.ascontiguousarray(W1[e][D:]).astype(BF16),
            "w2": W2[e].astype(BF16),
            "wq": np.ascontiguousarray(wq[:, dsl]),
            "wk": np.ascontiguousarray(wk[:, dsl]),
            "wr": wr,
            "b1c": np.ascontiguousarray(b1[e].reshape(HB, 128).T),
            "b2r": b2[e].reshape(1, D).astype(BF16),
            "bqc": np.ascontiguousarray(bq[dsl].reshape(128, 1)),
            "bkc": np.ascontiguousarray(bk[dsl].reshape(128, 1)),
            "brc": brc,
            "esel": sel.astype(BF16), "vmask": vm, "selbc": selbc_h,
        })
    return in_maps, m_idx


def _run(cfg, in_maps, trace=False, sim=False):
    global LAST_RESULT
    key = cfg["key"]
    if key not in _GRAPH_CACHE:
        _GRAPH_CACHE[key] = build_graph(cfg)
    nc = _GRAPH_CACHE[key]
    if sim:
        from concourse import bass_interp
        s = bass_interp.MultiCoreSim(nc, cfg["NC"])
        for c in range(cfg["NC"]):
            for k, v in in_maps[c].items():
                s.cores[c].tensor(k)[:] = v
        s.simulate(check_with_hw=False)
        return [{"out": np.asarray(s.cores[c].mem_tensor("out"))}
                for c in range(cfg["NC"])]
    from concourse import bass_utils
    kw = {}
    if trace and os.environ.get("KERNEL_TRACE_DIR"):
        kw["tmpdir"] = os.environ["KERNEL_TRACE_DIR"]
    res = bass_utils.run_bass_kernel_spmd(
        nc, in_maps, core_ids=list(range(cfg["NC"])), trace=trace, **kw,
    )
    LAST_RESULT = res
    return res.results


def kernel_impl(inputs, D, K, L, M, U, DP, H, NC, MCW, QCW, NRS, sim=False,
                hid_act="Gelu", SCW=None):
    PMAX = M * 10

    m_idx = np.asarray(inputs["mask_indices"]).astype(np.int64)
    u_idx = np.asarray(inputs["unmasked_indices"]).astype(np.int64)
    r = int(np.asarray(inputs["range_r"]))

    offs, valid = build_tables(m_idx, u_idx, r, PMAX)
    J = len(offs)
    if J == 0:
        return np.zeros((1, L, D), np.float32)
    PAD = int(max(8, np.max(np.abs(offs))))
    PAD = (PAD + 7) // 8 * 8
    b2z = not np.any(np.asarray(inputs["b2"]))

    cfg = {
        "D": D, "H": H, "M": M, "U": U, "DP": DP, "K": K, "NC": NC,
        "offs": offs, "valid": valid, "PAD": PAD, "b2z": b2z,
        "MCW": MCW, "QCW": QCW, "NRS": NRS, "hid_act": hid_act,
        "key": (D, H, M, U, DP, K, NC, MCW, QCW, NRS, PAD, hid_act, b2z,
                tuple(offs.tolist())),
    }

    in_maps, m_idx = _prepare(inputs, cfg)
    results = _run(cfg, in_maps, trace=bool(os.environ.get("KERNEL_TRACE")),
                   sim=sim)

    deltaT = np.concatenate(
        [np.asarray(results[c]["out"], np.float32) for c in range(NC)], axis=0
    )  # [D, M]
    delta_md = deltaT.T  # [M, D]
    out = np.zeros((L, D), np.float32)
    if len(np.unique(m_idx)) == len(m_idx):
        out[m_idx] = delta_md
    else:
        np.add.at(out, m_idx, delta_md)
    return out[None]


def kernel(**inputs):
    return kernel_impl(
        inputs, D=4096, K=8, L=2048, M=1024, U=1024, DP=512, H=2048,
        NC=NCORES, MCW=512, QCW=512, NRS=4,
    )


# revision 25
# speedup vs baseline: 1.1232x; 1.0426x over previous
"""Trainium2 Bass kernel for nn_AMIPRouterInference (gnn_message_passing).

v2: flat pool (no phase barriers), A-phase overlapped with score AllReduce +
softmax, in-place m1b->hid buffer merge, FD=512 vector ops, partition-parallel
softmax, SBUF-resident combine weights, NRS=4 reduce-scatter.
"""

import os
import numpy as np

NCORES = 8

_GRAPH_CACHE = {}
LAST_RESULT = None  # BassKernelResults of the most recent device run


def build_tables(m_idx, u_idx, r, pmax):
    M = len(m_idx)
    dists = np.abs(m_idx[:, None].astype(np.int64) - u_idx[None, :].astype(np.int64))
    adj = (dists > 0) & (dists <= r)
    pair_m, pair_u = np.nonzero(adj)  # row-major == jnp.nonzero order
    pair_m = pair_m[:pmax]
    pair_u = pair_u[:pmax]
    offs = np.unique(pair_u - pair_m).astype(np.int64)
    J = len(offs)
    valid = np.zeros((J, M), dtype=np.float32)
    for j, d in enumerate(offs):
        valid[j, pair_m[(pair_u - pair_m) == d]] = 1.0
    return offs, valid


def build_graph(cfg):
    import contextlib
    import concourse.mybir as mybir
    import concourse.tile as tile
    from concourse import bacc

    D, H, M, U, DP, K = cfg["D"], cfg["H"], cfg["M"], cfg["U"], cfg["DP"], cfg["K"]
    NC = cfg["NC"]
    offs = cfg["offs"]
    J = len(offs)
    PAD = cfg["PAD"]
    MCW = cfg["MCW"]            # compute chunk width along M
    NMC = M // MCW
    QCW = cfg["QCW"]            # qk/score-phase chunk width
    NQC = M // QCW
    DB, HB, DPB = D // 128, H // 128, DP // 128
    HGS = 2                     # h-blocks per A/M-phase psum group
    DGS = 4                     # d-blocks per W2-phase psum group
    DSLAB = 4                   # d-blocks per weight DMA slab
    HSLAB = 4                   # h-blocks per W2 weight DMA slab
    RSD = D // NC               # rows of final output per core
    NRS = cfg["NRS"]            # number of reduce-scatter column groups
    RSW = M // NRS
    b2z = cfg.get("b2z", False)
    assert M % MCW == 0 and M % QCW == 0 and M % NRS == 0 and MCW % RSW == 0

    bf16 = mybir.dt.bfloat16
    f32 = mybir.dt.float32
    AF = mybir.ActivationFunctionType
    hid_af = getattr(AF, cfg.get("hid_act", "Gelu"))

    nc = bacc.Bacc(None, target_bir_lowering=False, debug=False)

    # ---------------- DRAM parameters ----------------
    hmT = nc.declare_dram_parameter("hmT", [D, M], bf16, isOutput=False)
    huT = nc.declare_dram_parameter("huT", [D, U], bf16, isOutput=False)
    w1a = nc.declare_dram_parameter("w1a", [D, H], bf16, isOutput=False)
    w1b = nc.declare_dram_parameter("w1b", [D, H], bf16, isOutput=False)
    w2 = nc.declare_dram_parameter("w2", [H, D], bf16, isOutput=False)
    wq = nc.declare_dram_parameter("wq", [D, 128], bf16, isOutput=False)
    wk = nc.declare_dram_parameter("wk", [D, 128], bf16, isOutput=False)
    wr = nc.declare_dram_parameter("wr", [D, K], bf16, isOutput=False)
    b1c = nc.declare_dram_parameter("b1c", [128, HB], f32, isOutput=False)
    b2r = nc.declare_dram_parameter("b2r", [1, D], bf16, isOutput=False)
    bqc = nc.declare_dram_parameter("bqc", [128, 1], f32, isOutput=False)
    bkc = nc.declare_dram_parameter("bkc", [128, 1], f32, isOutput=False)
    brc = nc.declare_dram_parameter("brc", [128, 1], f32, isOutput=False)
    esel = nc.declare_dram_parameter("esel", [K, 1], bf16, isOutput=False)
    selbc = nc.declare_dram_parameter("selbc", [J, J * 128], bf16,
                                      isOutput=False)
    vmask = nc.declare_dram_parameter("vmask", [J, M], bf16, isOutput=False)
    outp = nc.declare_dram_parameter("out", [RSD, M], bf16, isOutput=True)

    with tile.TileContext(nc) as tc, contextlib.ExitStack() as ctx:
        sb = ctx.enter_context(tc.tile_pool(name="sb", bufs=1))
        ps = ctx.enter_context(tc.tile_pool(name="ps", bufs=1, space="PSUM"))
        dram = ctx.enter_context(tc.tile_pool(name="dram", bufs=1, space="DRAM"))

        def psum_mm(name):
            return ps.tile([128, 512], f32, tag="mm", bufs=6, name=name)

        def psum_row(name):
            return ps.tile([16, 512], f32, tag="row", bufs=2, name=name)

        # ---------------- persistent SBUF tensors ----------------
        ones = sb.tile([128, 128], bf16, name="ones")
        nc.vector.memset(ones[:, :], 1.0)
        ones32 = sb.tile([128, 1], f32, name="ones32")
        nc.vector.memset(ones32[:, :], 1.0)

        b1_sb = sb.tile([128, HB], f32, name="b1_sb")
        nc.sync.dma_start(b1_sb[:, :], b1c[:, :])
        bq_sb = sb.tile([128, 1], f32, name="bq_sb")
        nc.sync.dma_start(bq_sb[:, :], bqc[:, :])
        bk_sb = sb.tile([128, 1], f32, name="bk_sb")
        nc.sync.dma_start(bk_sb[:, :], bkc[:, :])
        br_sb = sb.tile([128, 1], f32, name="br_sb")
        nc.sync.dma_start(br_sb[:, :], brc[:, :])
        esel_sb = sb.tile([K, 1], bf16, name="esel_sb")
        nc.sync.dma_start(esel_sb[:, :], esel[:, :])
        wr_sb = sb.tile([128, DB, K], bf16, name="wr_sb")
        nc.sync.dma_start(
            wr_sb[:, :, :], wr.ap().rearrange("(o p) k -> p o k", p=128)
        )
        if not b2z:
            b2_sb = sb.tile([1, D], bf16, name="b2_sb")
            nc.sync.dma_start(b2_sb[:, :], b2r[:, :])
        vm_sb = sb.tile([J, M], bf16, name="vm_sb")
        nc.sync.dma_start(vm_sb[:, :], vmask[:, :])

        # big persistent tensors
        A1T = sb.tile([128, HB, U + 2 * PAD], bf16, name="A1T")
        for hb in range(HB):
            nc.vector.memset(A1T[:, hb, 0:PAD], 0.0)
            nc.vector.memset(A1T[:, hb, PAD + U: U + 2 * PAD], 0.0)

        def mh_tile(name):
            # holds M1 (post-bias) per chunk, overwritten in place by hid
            return sb.tile([128, HB, MCW], bf16, tag="mh", bufs=2, name=name)

        def hmc_tile(name):
            # streamed h chunks (hu for A phase, hm for M phase)
            return sb.tile([128, DB, MCW], bf16, tag="hmc", bufs=2, name=name)

        def w1t_tile(name):
            return sb.tile([128, DSLAB, HGS * 128], bf16, tag="w1t", bufs=3,
                           name=name)

        def w2t_tile(name):
            return sb.tile([128, HSLAB, DGS * 128], bf16, tag="w2t", bufs=3,
                           name=name)

        def cwb_tile(name):
            return sb.tile([128, J, MCW], bf16, tag="cwb", bufs=1, name=name)

        # qk / softmax phase tiles
        kT_sb = sb.tile([128, U + 2 * PAD], bf16, name="kT_sb")
        nc.vector.memset(kT_sb[:, 0:PAD], 0.0)
        nc.vector.memset(kT_sb[:, PAD + U: U + 2 * PAD], 0.0)
        qT_sb = sb.tile([128, M], bf16, name="qT_sb")
        eg_sb = sb.tile([K, M], bf16, name="eg_sb")
        ej = sb.tile([J, M], bf16, name="ej")
        r0 = sb.tile([1, M], bf16, name="r0")
        g0 = sb.tile([1, M], bf16, name="g0")
        g1 = sb.tile([1, M], bf16, name="g1")
        wbJ = sb.tile([J, M], bf16, name="wbJ")
        cwsum_bf = sb.tile([1, M], bf16, name="cwsum_bf") if not b2z else None
        # one-hot selectors: sel_bc[c, j*128+p] = (c == j), used as matmul lhsT
        # to broadcast row j of a [J, M] tile across 128 partitions
        sel_bc = sb.tile([J, J * 128], bf16, name="sel_bc")
        nc.sync.dma_start(sel_bc[:, :], selbc[:, :])

        sraw_b = dram.tile([J, M], f32, name="sraw_b")
        sred_b = dram.tile(
            [J, M], f32, name="sred_b",
            addr_space="Shared" if NC > 4 else "Local",
        )
        DBAND = D // NRS            # rows per reduce-scatter band
        DBPB = DB // NRS            # d-blocks per band
        bounce = [
            dram.tile([DBAND, M], bf16, name=f"bounce{g}") for g in range(NRS)
        ]
        rsout = [
            dram.tile([DBAND // NC, M], bf16, name=f"rsout{g}")
            for g in range(NRS)
        ]

        # ================= phase 1: q/k/gate + raw scores =================
        DBB = 2

        def qk_mm(ch):
            csl = slice(ch * QCW, (ch + 1) * QCW)
            psq = psum_mm("psq")
            psg = psum_row("psg")
            for dbb in range(0, DB, DBB):
                hm_t = sb.tile([128, DBB, QCW], bf16, tag="ht", bufs=2,
                               name="hm_t")
                nc.sync.dma_start(
                    hm_t[:, :, :],
                    hmT[dbb * 128:(dbb + DBB) * 128, csl].rearrange(
                        "(o p) m -> p o m", p=128),
                )
                wq_t = sb.tile([128, DBB, 128], bf16, tag="wt", bufs=2,
                               name="wq_t")
                nc.sync.dma_start(
                    wq_t[:, :, :],
                    wq[dbb * 128:(dbb + DBB) * 128, :].rearrange(
                        "(o p) m -> p o m", p=128),
                )
                for i in range(DBB):
                    db = dbb + i
                    st, sp = db == 0, db == DB - 1
                    nc.tensor.matmul(
                        psq[:, :QCW], wq_t[:, i, :], hm_t[:, i, :],
                        start=st, stop=sp,
                    )
                    nc.tensor.matmul(
                        psg[:K, :QCW], wr_sb[:, db, :], hm_t[:, i, :],
                        start=st, stop=sp,
                    )
            nc.vector.tensor_scalar_add(
                qT_sb[:, csl], psq[:, :QCW], bq_sb[:, 0:1],
            )
            nc.scalar.activation(
                eg_sb[:, csl], psg[:K, :QCW], AF.Exp,
                bias=br_sb[0:K, 0:1], scale=1.0,
            )
            psk = psum_mm("psk")
            for dbb in range(0, DB, DBB):
                hu_t = sb.tile([128, DBB, QCW], bf16, tag="ht", bufs=2,
                               name="hu_t")
                nc.sync.dma_start(
                    hu_t[:, :, :],
                    huT[dbb * 128:(dbb + DBB) * 128, csl].rearrange(
                        "(o p) m -> p o m", p=128),
                )
                wk_t = sb.tile([128, DBB, 128], bf16, tag="wt", bufs=2,
                               name="wk_t")
                nc.sync.dma_start(
                    wk_t[:, :, :],
                    wk[dbb * 128:(dbb + DBB) * 128, :].rearrange(
                        "(o p) m -> p o m", p=128),
                )
                for i in range(DBB):
                    db = dbb + i
                    nc.tensor.matmul(
                        psk[:, :QCW], wk_t[:, i, :], hu_t[:, i, :],
                        start=db == 0, stop=db == DB - 1,
                    )
            nc.vector.tensor_scalar_add(
                kT_sb[:, PAD + ch * QCW: PAD + (ch + 1) * QCW],
                psk[:, :QCW], bk_sb[:, 0:1],
            )

        def scores_all():
            for ch in range(NQC):
                qk_scores(ch)
            nc.gpsimd.collective_compute(
                "AllReduce",
                mybir.AluOpType.add,
                ins=[sraw_b.opt()],
                outs=[sred_b.opt()],
                replica_groups=[list(range(NC))],
            )

        def qk_scores(ch):
            for j in range(J):
                off = int(offs[j])
                pss = psum_row(f"pss{j}")
                prod = sb.tile([128, QCW], bf16, tag="prod", bufs=2,
                               name="prod")
                nc.vector.tensor_mul(
                    out=prod[:, :],
                    in0=qT_sb[:, ch * QCW:(ch + 1) * QCW],
                    in1=kT_sb[:, PAD + off + ch * QCW:
                              PAD + off + (ch + 1) * QCW],
                )
                nc.tensor.matmul(
                    pss[0:1, :QCW], ones[:, 0:1], prod[:, :],
                    start=True, stop=True,
                )
                s_t = sb.tile([1, QCW], f32, tag="st", bufs=1, name="s_t")
                nc.vector.tensor_copy(s_t[:, :], pss[0:1, :QCW])
                nc.gpsimd.dma_start(
                    sraw_b[j:j + 1, ch * QCW:(ch + 1) * QCW], s_t[:, :]
                )
        # ========== A phase: A1 = W1a^T @ hu (chunk-major, overlaps AR) =====
        def a_chunk(chx, mid_hook=None):
            hu_c = hmc_tile(f"hu{chx}")
            nc.sync.dma_start(
                hu_c[:, :, :],
                huT.ap().rearrange("(o p) m -> p o m", p=128)[
                    :, :, chx * MCW:(chx + 1) * MCW],
            )
            for hg in range(0, HB, HGS):
                if hg == 2 * HGS and mid_hook is not None:
                    mid_hook()
                psa = [psum_mm(f"psa{gi}") for gi in range(HGS)]
                for dbb in range(0, DB, DSLAB):
                    w_t = w1t_tile("w1a_t")
                    nc.sync.dma_start(
                        w_t[:, :, :],
                        w1a[dbb * 128:(dbb + DSLAB) * 128,
                            hg * 128:(hg + HGS) * 128].rearrange(
                                "(o p) h -> p o h", p=128),
                    )
                    for i in range(DSLAB):
                        db = dbb + i
                        for gi in range(HGS):
                            nc.tensor.matmul(
                                psa[gi][:, :MCW],
                                w_t[:, i, gi * 128:(gi + 1) * 128],
                                hu_c[:, db, :],
                                start=db == 0, stop=db == DB - 1,
                            )
                for gi in range(HGS):
                    nc.scalar.activation(
                        A1T[:, hg + gi,
                            PAD + chx * MCW: PAD + (chx + 1) * MCW],
                        psa[gi][:, :MCW], AF.Identity,
                        bias=0.0, scale=1.0,
                    )

        # interleave: qk(0), A(0), qk(1), A(1) with scores+AllReduce emitted
        # after A(1)'s first h-groups so score matmuls don't head-block A(1)
        qk_mm(0)
        a_chunk(0)
        for ch in range(1, NQC):
            qk_mm(ch)
        if NMC > 1:
            for chx in range(1, NMC):
                a_chunk(chx, mid_hook=scores_all if chx == 1 else None)
        else:
            scores_all()

        # ========== phase 2: softmax / combine (overlaps A phase) ==========
        ncopies = max(1, NC // max(1, DPB))
        inv_sqrt_dp = 1.0 / (float(np.sqrt(DP)) * ncopies)
        sr2 = sb.tile([J, M], f32, name="sr2")
        nc.gpsimd.dma_start(sr2[:, :], sred_b[:, :])
        nc.scalar.activation(
            ej[:, :], sr2[:, :], AF.Exp, bias=0.0, scale=inv_sqrt_dp,
        )
        nc.vector.tensor_mul(out=ej[:, :], in0=ej[:, :], in1=vm_sb[:, :])
        # r0 = 1/max(sum_j ej, eps); g0 = 1/sum_k eg; g1 = eg[esel]
        for half in range(M // 512):
            hsl = slice(half * 512, (half + 1) * 512)
            p1 = psum_row("p_rsum")
            nc.tensor.matmul(p1[0:1, :], ones[0:J, 0:1], ej[:, hsl],
                             start=True, stop=True)
            nc.vector.tensor_copy(r0[0:1, hsl], p1[0:1, :])
            p2 = psum_row("p_gsum")
            nc.tensor.matmul(p2[0:1, :], ones[0:K, 0:1], eg_sb[:, hsl],
                             start=True, stop=True)
            nc.vector.tensor_copy(g0[0:1, hsl], p2[0:1, :])
            p3 = psum_row("p_gsel")
            nc.tensor.matmul(p3[0:1, :], esel_sb[:, 0:1], eg_sb[:, hsl],
                             start=True, stop=True)
            nc.vector.tensor_copy(g1[0:1, hsl], p3[0:1, :])
        if not b2z:
            nc.vector.tensor_copy(cwsum_bf[0:1, :], r0[0:1, :])  # rowsum
        nc.vector.tensor_scalar_max(r0[0:1, :], r0[0:1, :], 1e-8)
        with nc.allow_low_precision(reason="bf16 softmax denominators"):
            nc.vector.reciprocal(r0[0:1, :], r0[0:1, :])
            nc.vector.reciprocal(g0[0:1, :], g0[0:1, :])
        nc.vector.tensor_mul(out=g1[0:1, :], in0=g1[0:1, :], in1=g0[0:1, :])
        # w = gate[e] / rowsum, broadcast to [J, M]; fold into ej in place
        nc.vector.tensor_mul(out=g1[0:1, :], in0=g1[0:1, :], in1=r0[0:1, :])
        if not b2z:
            # cwsum = rowsum * w  (~gate[e], but 0 for pairless masks)
            nc.vector.tensor_mul(out=cwsum_bf[0:1, :], in0=cwsum_bf[0:1, :],
                                 in1=g1[0:1, :])
        for half in range(M // 512):
            hsl = slice(half * 512, (half + 1) * 512)
            pw = psum_row("p_wbj")
            nc.tensor.matmul(pw[0:J, :], ones[0:1, 0:J], g1[0:1, hsl],
                             start=True, stop=True)
            nc.vector.tensor_copy(wbJ[:, hsl], pw[0:J, :])
        nc.vector.tensor_mul(out=ej[:, :], in0=ej[:, :], in1=wbJ[:, :])

        # cwb: broadcast combine-weight rows across partitions (built lazily
        # per chunk so the single buffer slot never inverts engine order)
        def cwb_build(mc):
            cwb = cwb_tile(f"cwb{mc}")
            for j in range(J):
                psb = psum_mm(f"psb{j}")
                nc.tensor.matmul(
                    psb[:, :MCW], sel_bc[:, j * 128:(j + 1) * 128],
                    ej[:, mc * MCW:(mc + 1) * MCW],
                    start=True, stop=True,
                )
                nc.vector.tensor_copy(cwb[:, j, :], psb[:, :MCW])
            return cwb

        # ================= steady pipeline: M -> hid -> W2 =================
        def m_phase(mc, mh_t):
            hm_c = hmc_tile(f"hm{mc}")
            nc.sync.dma_start(
                hm_c[:, :, :],
                hmT.ap().rearrange("(o p) m -> p o m", p=128)[
                    :, :, mc * MCW:(mc + 1) * MCW],
            )
            for hg in range(0, HB, HGS):
                psm = [psum_mm(f"psm{gi}") for gi in range(HGS)]
                for dbb in range(0, DB, DSLAB):
                    w_t = w1t_tile("w1b_t")
                    nc.sync.dma_start(
                        w_t[:, :, :],
                        w1b[dbb * 128:(dbb + DSLAB) * 128,
                            hg * 128:(hg + HGS) * 128].rearrange(
                                "(o p) h -> p o h", p=128),
                    )
                    for i in range(DSLAB):
                        db = dbb + i
                        for gi in range(HGS):
                            nc.tensor.matmul(
                                psm[gi][:, :MCW],
                                w_t[:, i, gi * 128:(gi + 1) * 128],
                                hm_c[:, db, :],
                                start=db == 0, stop=db == DB - 1,
                            )
                for gi in range(HGS):
                    hb = hg + gi
                    nc.scalar.activation(
                        mh_t[:, hb, :], psm[gi][:, :MCW], AF.Identity,
                        bias=b1_sb[:, hb:hb + 1], scale=1.0,
                    )

        def hid_phase(mc, mh_t, cwb):
            c0 = mc * MCW
            for hb in range(HB):
                scr = sb.tile([128, MCW], bf16, tag="scr", bufs=2, name="scr")
                for j in range(J):
                    off = int(offs[j])
                    x_t = sb.tile([128, MCW], bf16, tag="xt", bufs=2,
                                  name="x_t")
                    nc.vector.tensor_add(
                        out=x_t[:, :],
                        in0=A1T[:, hb, PAD + off + c0: PAD + off + c0 + MCW],
                        in1=mh_t[:, hb, :],
                    )
                    g_t = sb.tile([128, MCW], bf16, tag="gt", bufs=2,
                                  name="g_t")
                    nc.scalar.activation(
                        g_t[:, :], x_t[:, :], hid_af, bias=0.0, scale=1.0,
                    )
                    if j == 0:
                        dst = scr[:, :] if J > 1 else mh_t[:, hb, :]
                        nc.vector.tensor_mul(
                            out=dst, in0=g_t[:, :], in1=cwb[:, j, :],
                        )
                    elif j < J - 1:
                        nc.vector.tensor_mul(
                            out=g_t[:, :], in0=g_t[:, :], in1=cwb[:, j, :]
                        )
                        nc.vector.tensor_add(
                            out=scr[:, :], in0=scr[:, :], in1=g_t[:, :]
                        )
                    else:
                        nc.vector.tensor_mul(
                            out=g_t[:, :], in0=g_t[:, :], in1=cwb[:, j, :]
                        )
                        nc.vector.tensor_add(
                            out=mh_t[:, hb, :], in0=scr[:, :], in1=g_t[:, :]
                        )

        def w2_pass(mc, mh_t, c0, cw, fire_rs=False):
            lo = c0 - mc * MCW
            for dg in range(0, DB, DGS):
                psd = [psum_mm(f"psd{gi}") for gi in range(DGS)]
                for hbb in range(0, HB, HSLAB):
                    w2_t = w2t_tile("w2_t")
                    nc.sync.dma_start(
                        w2_t[:, :, :],
                        w2[hbb * 128:(hbb + HSLAB) * 128,
                           dg * 128:(dg + DGS) * 128].rearrange(
                               "(o p) d -> p o d", p=128),
                    )
                    for i in range(HSLAB):
                        hb = hbb + i
                        for gi in range(DGS):
                            nc.tensor.matmul(
                                psd[gi][:, :cw],
                                w2_t[:, i, gi * 128:(gi + 1) * 128],
                                mh_t[:, hb, lo:lo + cw],
                                start=hb == 0,
                                stop=(hb == HB - 1) if b2z else False,
                            )
                for gi in range(DGS):
                    db = dg + gi
                    if not b2z:
                        nc.tensor.matmul(
                            psd[gi][:, :cw],
                            b2_sb[0:1, db * 128:(db + 1) * 128],
                            cwsum_bf[0:1, c0:c0 + cw],
                            start=False, stop=True,
                        )
                    d_t = sb.tile([128, MCW], bf16, tag="dt", bufs=2,
                                  name="d_t")
                    nc.scalar.activation(
                        d_t[:, :cw], psd[gi][:, :cw], AF.Identity,
                        bias=0.0, scale=1.0,
                    )
                    band = db // DBPB
                    row0 = (db % DBPB) * 128
                    nc.sync.dma_start(
                        bounce[band][row0:row0 + 128, c0:c0 + cw],
                        d_t[:, :cw],
                    )
                if fire_rs and (dg + DGS) % DBPB == 0:
                    rs_group(dg // DBPB)

        def rs_group(g):
            nc.gpsimd.collective_compute(
                "ReduceScatter",
                mybir.AluOpType.add,
                ins=[bounce[g].opt()],
                outs=[rsout[g].opt()],
                replica_groups=[list(range(NC))],
            )
            nc.gpsimd.dma_start(
                outp[g * (DBAND // NC):(g + 1) * (DBAND // NC), :],
                rsout[g][:, :],
            )

        mh_store = {}
        for mc in range(NMC):
            mh_store[mc] = mh_tile(f"mh{mc}")
            m_phase(mc, mh_store[mc])
            cwb_mc = cwb_build(mc)
            hid_phase(mc, mh_store[mc], cwb_mc)
            if mc >= 1:
                pm = mc - 1
                w2_pass(pm, mh_store[pm], pm * MCW, MCW,
                        fire_rs=(pm == NMC - 1))
                del mh_store[pm]
        mc = NMC - 1
        w2_pass(mc, mh_store[mc], mc * MCW, MCW, fire_rs=True)

    nc.finalize()
    return nc


def _prepare(inputs, cfg):
    import ml_dtypes
    BF16 = ml_dtypes.bfloat16
    D, H, M, U, DP, K = cfg["D"], cfg["H"], cfg["M"], cfg["U"], cfg["DP"], cfg["K"]
    HB, DPB = H // 128, DP // 128
    offs, valid = cfg["offs"], cfg["valid"]
    J = len(offs)

    h = np.asarray(inputs["h_L"], dtype=np.float32)[0]
    m_idx = np.asarray(inputs["mask_indices"]).astype(np.int64)
    u_idx = np.asarray(inputs["unmasked_indices"]).astype(np.int64)

    hmT = np.ascontiguousarray(h[m_idx].astype(BF16).T)
    huT = np.ascontiguousarray(h[u_idx].astype(BF16).T)
    wq = np.asarray(inputs["Wq"], np.float32).astype(BF16)
    wk = np.asarray(inputs["Wk"], np.float32).astype(BF16)
    wr = np.asarray(inputs["Wr"], np.float32).astype(BF16)
    bq = np.asarray(inputs["bq"], np.float32)
    bk = np.asarray(inputs["bk"], np.float32)
    brc = np.zeros((128, 1), np.float32)
    brc[:K, 0] = np.asarray(inputs["br"], np.float32)
    vm = np.ascontiguousarray(valid).astype(BF16)  # [J, M]

    W1 = np.asarray(inputs["W1"], np.float32)
    W2 = np.asarray(inputs["W2"], np.float32)
    b1 = np.asarray(inputs["b1"], np.float32)
    b2 = np.asarray(inputs["b2"], np.float32)

    selbc_h = np.zeros((J, J * 128), dtype=BF16)
    for j in range(J):
        selbc_h[j, j * 128:(j + 1) * 128] = 1.0

    DPBT = max(1, DP // 128)
    in_maps = []
    for c in range(cfg["NC"]):
        e = c % K
        dpb = c % DPBT
        dsl = slice(dpb * 128, (dpb + 1) * 128)
        sel = np.zeros((K, 1), np.float32)
        sel[e, 0] = 1.0
        in_maps.append({
            "hmT": hmT, "huT": huT,
            "w1a": np.ascontiguousarray(W1[e][:D]).astype(BF16),
            "w1b": np.ascontiguousarray(W1[e][D:]).astype(BF16),
            "w2": W2[e].astype(BF16),
            "wq": np.ascontiguousarray(wq[:, dsl]),
            "wk": np.ascontiguousarray(wk[:, dsl]),
            "wr": wr,
            "b1c": np.ascontiguousarray(b1[e].reshape(HB, 128).T),
            "b2r": b2[e].reshape(1, D).astype(BF16),
            "bqc": np.ascontiguousarray(bq[dsl].reshape(128, 1)),
            "bkc": np.ascontiguousarray(bk[dsl].reshape(128, 1)),
            "brc": brc,
            "esel": sel.astype(BF16), "vmask": vm, "selbc": selbc_h,
        })
    return in_maps, m_idx


def _run(cfg, in_maps, trace=False, sim=False):
    global LAST_RESULT
    key = cfg["key"]
    if key not in _GRAPH_CACHE:
        _GRAPH_CACHE[key] = build_graph(cfg)
    nc = _GRAPH_CACHE[key]
    if sim:
        from concourse import bass_interp
        s = bass_interp.MultiCoreSim(nc, cfg["NC"])
        for c in range(cfg["NC"]):
            for k, v in in_maps[c].items():
                s.cores[c].tensor(k)[:] = v
        s.simulate(check_with_hw=False)
        return [{"out": np.asarray(s.cores[c].mem_tensor("out"))}
                for c in range(cfg["NC"])]
    from concourse import bass_utils
    kw = {}
    if trace and os.environ.get("KERNEL_TRACE_DIR"):
        kw["tmpdir"] = os.environ["KERNEL_TRACE_DIR"]
    res = bass_utils.run_bass_kernel_spmd(
        nc, in_maps, core_ids=list(range(cfg["NC"])), trace=trace, **kw,
    )
    LAST_RESULT = res
    return res.results


def kernel_impl(inputs, D, K, L, M, U, DP, H, NC, MCW, QCW, NRS, sim=False,
                hid_act="Gelu", SCW=None):
    PMAX = M * 10

    m_idx = np.asarray(inputs["mask_indices"]).astype(np.int64)
    u_idx = np.asarray(inputs["unmasked_indices"]).astype(np.int64)
    r = int(np.asarray(inputs["range_r"]))

    offs, valid = build_tables(m_idx, u_idx, r, PMAX)
    J = len(offs)
    if J == 0:
        return np.zeros((1, L, D), np.float32)
    PAD = int(max(8, np.max(np.abs(offs))))
    PAD = (PAD + 7) // 8 * 8
    b2z = not np.any(np.asarray(inputs["b2"]))

    cfg = {
        "D": D, "H": H, "M": M, "U": U, "DP": DP, "K": K, "NC": NC,
        "offs": offs, "valid": valid, "PAD": PAD, "b2z": b2z,
        "MCW": MCW, "QCW": QCW, "NRS": NRS, "hid_act": hid_act,
        "key": (D, H, M, U, DP, K, NC, MCW, QCW, NRS, PAD, hid_act, b2z,
                tuple(offs.tolist())),
    }

    in_maps, m_idx = _prepare(inputs, cfg)
    results = _run(cfg, in_maps, trace=bool(os.environ.get("KERNEL_TRACE")),
                   sim=sim)

    # cores hold D-band-interleaved rows: core c, band g covers absolute
    # d-rows [g*DBAND + c*BR, g*DBAND + (c+1)*BR)
    DBAND = D // NRS
    BR = DBAND // NC
    deltaT = np.empty((D, M), np.float32)
    for c in range(NC):
        oc = np.asarray(results[c]["out"], np.float32)  # [D//NC, M]
        for g in range(NRS):
            deltaT[g * DBAND + c * BR: g * DBAND + (c + 1) * BR] = \
                oc[g * BR:(g + 1) * BR]
    delta_md = deltaT.T  # [M, D]
    out = np.zeros((L, D), np.float32)
    if len(np.unique(m_idx)) == len(m_idx):
        out[m_idx] = delta_md
    else:
        np.add.at(out, m_idx, delta_md)
    return out[None]


def kernel(**inputs):
    return kernel_impl(
        inputs, D=4096, K=8, L=2048, M=1024, U=1024, DP=512, H=2048,
        NC=NCORES, MCW=512, QCW=512, NRS=4,
    )


# revision 28
# speedup vs baseline: 1.2259x; 1.0915x over previous
"""Trainium2 Bass kernel for nn_AMIPRouterInference (gnn_message_passing).

v2: flat pool (no phase barriers), A-phase overlapped with score AllReduce +
softmax, in-place m1b->hid buffer merge, FD=512 vector ops, partition-parallel
softmax, SBUF-resident combine weights, NRS=4 reduce-scatter.
"""

import os
import numpy as np

NCORES = 8

_GRAPH_CACHE = {}
LAST_RESULT = None  # BassKernelResults of the most recent device run


def build_tables(m_idx, u_idx, r, pmax):
    M = len(m_idx)
    dists = np.abs(m_idx[:, None].astype(np.int64) - u_idx[None, :].astype(np.int64))
    adj = (dists > 0) & (dists <= r)
    pair_m, pair_u = np.nonzero(adj)  # row-major == jnp.nonzero order
    pair_m = pair_m[:pmax]
    pair_u = pair_u[:pmax]
    offs = np.unique(pair_u - pair_m).astype(np.int64)
    J = len(offs)
    valid = np.zeros((J, M), dtype=np.float32)
    for j, d in enumerate(offs):
        valid[j, pair_m[(pair_u - pair_m) == d]] = 1.0
    return offs, valid


def build_graph(cfg):
    import contextlib
    import concourse.mybir as mybir
    import concourse.tile as tile
    from concourse import bacc
    from concourse import bass_isa

    D, H, M, U, DP, K = cfg["D"], cfg["H"], cfg["M"], cfg["U"], cfg["DP"], cfg["K"]
    NC = cfg["NC"]
    offs = cfg["offs"]
    J = len(offs)
    PAD = cfg["PAD"]
    MCW = cfg["MCW"]            # compute chunk width along M
    NMC = M // MCW
    QCW = cfg["QCW"]            # qk/score-phase chunk width
    NQC = M // QCW
    DB, HB, DPB = D // 128, H // 128, DP // 128
    HGS = 2                     # h-blocks per A/M-phase psum group
    DGS = 4                     # d-blocks per W2-phase psum group
    DSLAB = 4                   # d-blocks per weight DMA slab
    HSLAB = 4                   # h-blocks per W2 weight DMA slab
    RSD = D // NC               # rows of final output per core
    NRS = cfg["NRS"]            # number of reduce-scatter column groups
    RSW = M // NRS
    b2z = cfg.get("b2z", False)
    assert M % MCW == 0 and M % QCW == 0 and M % NRS == 0 and MCW % RSW == 0

    bf16 = mybir.dt.bfloat16
    f32 = mybir.dt.float32
    AF = mybir.ActivationFunctionType
    hid_af = getattr(AF, cfg.get("hid_act", "Gelu"))

    nc = bacc.Bacc(None, target_bir_lowering=False, debug=False)

    # ---------------- DRAM parameters ----------------
    hmT = nc.declare_dram_parameter("hmT", [D, M], bf16, isOutput=False)
    huT = nc.declare_dram_parameter("huT", [D, U], bf16, isOutput=False)
    w1a = nc.declare_dram_parameter("w1a", [D, H], bf16, isOutput=False)
    w1b = nc.declare_dram_parameter("w1b", [D, H], bf16, isOutput=False)
    w2 = nc.declare_dram_parameter("w2", [H, D], bf16, isOutput=False)
    wq = nc.declare_dram_parameter("wq", [D, 128], bf16, isOutput=False)
    wk = nc.declare_dram_parameter("wk", [D, 128], bf16, isOutput=False)
    wr = nc.declare_dram_parameter("wr", [D, K], bf16, isOutput=False)
    b1c = nc.declare_dram_parameter("b1c", [128, HB], f32, isOutput=False)
    b2r = nc.declare_dram_parameter("b2r", [1, D], bf16, isOutput=False)
    bqc = nc.declare_dram_parameter("bqc", [128, 1], f32, isOutput=False)
    bkc = nc.declare_dram_parameter("bkc", [128, 1], f32, isOutput=False)
    brc = nc.declare_dram_parameter("brc", [128, 1], f32, isOutput=False)
    esel = nc.declare_dram_parameter("esel", [K, 1], f32, isOutput=False)
    vmask = nc.declare_dram_parameter("vmask", [J, M], bf16, isOutput=False)
    outp = nc.declare_dram_parameter("out", [RSD, M], bf16, isOutput=True)

    with tile.TileContext(nc) as tc, contextlib.ExitStack() as ctx:
        sb = ctx.enter_context(tc.tile_pool(name="sb", bufs=1))
        ps = ctx.enter_context(tc.tile_pool(name="ps", bufs=1, space="PSUM"))
        dram = ctx.enter_context(tc.tile_pool(name="dram", bufs=1, space="DRAM"))

        def psum_mm(name):
            return ps.tile([128, 512], f32, tag="mm", bufs=6, name=name)

        def psum_row(name):
            return ps.tile([16, 512], f32, tag="row", bufs=2, name=name)

        # ---------------- persistent SBUF tensors ----------------
        ones = sb.tile([128, 128], bf16, name="ones")
        nc.vector.memset(ones[:, :], 1.0)
        ones32 = sb.tile([128, 1], f32, name="ones32")
        nc.vector.memset(ones32[:, :], 1.0)

        b1_sb = sb.tile([128, HB], f32, name="b1_sb")
        nc.sync.dma_start(b1_sb[:, :], b1c[:, :])
        bq_sb = sb.tile([128, 1], f32, name="bq_sb")
        nc.sync.dma_start(bq_sb[:, :], bqc[:, :])
        bk_sb = sb.tile([128, 1], f32, name="bk_sb")
        nc.sync.dma_start(bk_sb[:, :], bkc[:, :])
        br_sb = sb.tile([128, 1], f32, name="br_sb")
        nc.sync.dma_start(br_sb[:, :], brc[:, :])
        esel_sb = sb.tile([K, 1], f32, name="esel_sb")
        nc.sync.dma_start(esel_sb[:, :], esel[:, :])
        wr_sb = sb.tile([128, DB, K], bf16, name="wr_sb")
        nc.sync.dma_start(
            wr_sb[:, :, :], wr.ap().rearrange("(o p) k -> p o k", p=128)
        )
        if not b2z:
            b2_sb = sb.tile([1, D], bf16, name="b2_sb")
            nc.sync.dma_start(b2_sb[:, :], b2r[:, :])
        vm_sb = sb.tile([J, M], bf16, tag="jm", bufs=1, name="vm_sb")
        nc.sync.dma_start(vm_sb[:, :], vmask[:, :])

        # big persistent tensors
        A1T = sb.tile([128, HB, U + 2 * PAD], bf16, name="A1T")
        for hb in range(HB):
            nc.vector.memset(A1T[:, hb, 0:PAD], 0.0)
            nc.vector.memset(A1T[:, hb, PAD + U: U + 2 * PAD], 0.0)

        def mh_tile(name):
            # holds M1 (post-bias) per chunk, overwritten in place by hid
            return sb.tile([128, HB, MCW], bf16, tag="mh", bufs=2, name=name)

        def hmc_tile(name):
            # streamed h chunks (hu for A phase, hm for M phase)
            return sb.tile([128, DB, MCW], bf16, tag="hmc", bufs=2, name=name)

        def w1t_tile(name):
            return sb.tile([128, DSLAB, HGS * 128], bf16, tag="w1t", bufs=3,
                           name=name)

        def w2t_tile(name):
            return sb.tile([128, HSLAB, DGS * 128], bf16, tag="w2t", bufs=3,
                           name=name)

        def cwb_tile(name):
            return sb.tile([128, J, MCW], bf16, tag="cwb", bufs=1, name=name)

        # qk / softmax phase tiles
        kT_sb = sb.tile([128, U + 2 * PAD], bf16, name="kT_sb")
        nc.vector.memset(kT_sb[:, 0:PAD], 0.0)
        nc.vector.memset(kT_sb[:, PAD + U: U + 2 * PAD], 0.0)
        qT_sb = sb.tile([128, M], bf16, name="qT_sb")
        eg_sb = sb.tile([K, M], bf16, name="eg_sb")
        ej = sb.tile([J, M], bf16, name="ej")
        r0 = sb.tile([1, M], bf16, name="r0")
        g0 = sb.tile([1, M], bf16, name="g0")
        g1 = sb.tile([1, M], bf16, name="g1")
        wbJ = sb.tile([J, M], bf16, name="wbJ")
        cwsum_bf = sb.tile([1, M], bf16, name="cwsum_bf") if not b2z else None

        sraw_b = dram.tile([J, M], f32, name="sraw_b")
        sred_b = dram.tile(
            [J, M], f32, name="sred_b",
            addr_space="Shared" if NC > 4 else "Local",
        )
        DBAND = D // NRS            # rows per reduce-scatter band
        DBPB = DB // NRS            # d-blocks per band
        bounce = [
            dram.tile([DBAND, M], bf16, name=f"bounce{g}") for g in range(NRS)
        ]
        rsout = [
            dram.tile([DBAND // NC, M], bf16, name=f"rsout{g}")
            for g in range(NRS)
        ]

        # ================= phase 1: q/k/gate + raw scores =================
        DBB = 2

        def qk_mm(ch):
            csl = slice(ch * QCW, (ch + 1) * QCW)
            psq = psum_mm("psq")
            psg = psum_row("psg")
            for dbb in range(0, DB, DBB):
                hm_t = sb.tile([128, DBB, QCW], bf16, tag="ht", bufs=2,
                               name="hm_t")
                nc.sync.dma_start(
                    hm_t[:, :, :],
                    hmT[dbb * 128:(dbb + DBB) * 128, csl].rearrange(
                        "(o p) m -> p o m", p=128),
                )
                wq_t = sb.tile([128, DBB, 128], bf16, tag="wt", bufs=2,
                               name="wq_t")
                nc.sync.dma_start(
                    wq_t[:, :, :],
                    wq[dbb * 128:(dbb + DBB) * 128, :].rearrange(
                        "(o p) m -> p o m", p=128),
                )
                for i in range(DBB):
                    db = dbb + i
                    st, sp = db == 0, db == DB - 1
                    nc.tensor.matmul(
                        psq[:, :QCW], wq_t[:, i, :], hm_t[:, i, :],
                        start=st, stop=sp,
                    )
                    nc.tensor.matmul(
                        psg[:K, :QCW], wr_sb[:, db, :], hm_t[:, i, :],
                        start=st, stop=sp,
                    )
            nc.vector.tensor_scalar_add(
                qT_sb[:, csl], psq[:, :QCW], bq_sb[:, 0:1],
            )
            nc.scalar.activation(
                eg_sb[:, csl], psg[:K, :QCW], AF.Exp,
                bias=br_sb[0:K, 0:1], scale=1.0,
            )
            psk = psum_mm("psk")
            for dbb in range(0, DB, DBB):
                hu_t = sb.tile([128, DBB, QCW], bf16, tag="ht", bufs=2,
                               name="hu_t")
                nc.sync.dma_start(
                    hu_t[:, :, :],
                    huT[dbb * 128:(dbb + DBB) * 128, csl].rearrange(
                        "(o p) m -> p o m", p=128),
                )
                wk_t = sb.tile([128, DBB, 128], bf16, tag="wt", bufs=2,
                               name="wk_t")
                nc.sync.dma_start(
                    wk_t[:, :, :],
                    wk[dbb * 128:(dbb + DBB) * 128, :].rearrange(
                        "(o p) m -> p o m", p=128),
                )
                for i in range(DBB):
                    db = dbb + i
                    nc.tensor.matmul(
                        psk[:, :QCW], wk_t[:, i, :], hu_t[:, i, :],
                        start=db == 0, stop=db == DB - 1,
                    )
            nc.vector.tensor_scalar_add(
                kT_sb[:, PAD + ch * QCW: PAD + (ch + 1) * QCW],
                psk[:, :QCW], bk_sb[:, 0:1],
            )

        def scores_all():
            for ch in range(NQC):
                qk_scores(ch)
            nc.gpsimd.collective_compute(
                "AllReduce",
                mybir.AluOpType.add,
                ins=[sraw_b.opt()],
                outs=[sred_b.opt()],
                replica_groups=[list(range(NC))],
            )

        def qk_scores(ch):
            for j in range(J):
                off = int(offs[j])
                pss = psum_row(f"pss{j}")
                prod = sb.tile([128, QCW], bf16, tag="prod", bufs=2,
                               name="prod")
                nc.vector.tensor_mul(
                    out=prod[:, :],
                    in0=qT_sb[:, ch * QCW:(ch + 1) * QCW],
                    in1=kT_sb[:, PAD + off + ch * QCW:
                              PAD + off + (ch + 1) * QCW],
                )
                nc.tensor.matmul(
                    pss[0:1, :QCW], ones[:, 0:1], prod[:, :],
                    start=True, stop=True,
                )
                s_t = sb.tile([1, QCW], f32, tag="st", bufs=1, name="s_t")
                nc.vector.tensor_copy(s_t[:, :], pss[0:1, :QCW])
                nc.gpsimd.dma_start(
                    sraw_b[j:j + 1, ch * QCW:(ch + 1) * QCW], s_t[:, :]
                )
        # ========== A phase: A1 = W1a^T @ hu (chunk-major, overlaps AR) =====
        def a_chunk(chx, mid_hook=None):
            hu_c = hmc_tile(f"hu{chx}")
            nc.sync.dma_start(
                hu_c[:, :, :],
                huT.ap().rearrange("(o p) m -> p o m", p=128)[
                    :, :, chx * MCW:(chx + 1) * MCW],
            )
            for hg in range(0, HB, HGS):
                if hg == 2 * HGS and mid_hook is not None:
                    mid_hook()
                psa = [psum_mm(f"psa{gi}") for gi in range(HGS)]
                for dbb in range(0, DB, DSLAB):
                    w_t = w1t_tile("w1a_t")
                    nc.sync.dma_start(
                        w_t[:, :, :],
                        w1a[dbb * 128:(dbb + DSLAB) * 128,
                            hg * 128:(hg + HGS) * 128].rearrange(
                                "(o p) h -> p o h", p=128),
                    )
                    for i in range(DSLAB):
                        db = dbb + i
                        for gi in range(HGS):
                            nc.tensor.matmul(
                                psa[gi][:, :MCW],
                                w_t[:, i, gi * 128:(gi + 1) * 128],
                                hu_c[:, db, :],
                                start=db == 0, stop=db == DB - 1,
                            )
                for gi in range(HGS):
                    nc.scalar.activation(
                        A1T[:, hg + gi,
                            PAD + chx * MCW: PAD + (chx + 1) * MCW],
                        psa[gi][:, :MCW], AF.Identity,
                        bias=0.0, scale=1.0,
                    )

        # interleave: qk(0), A(0), qk(1), A(1) with scores+AllReduce emitted
        # after A(1)'s first h-groups so score matmuls don't head-block A(1)
        qk_mm(0)
        a_chunk(0)
        for ch in range(1, NQC):
            qk_mm(ch)
        if NMC > 1:
            for chx in range(1, NMC):
                a_chunk(chx, mid_hook=scores_all if chx == 1 else None)
        else:
            scores_all()

        # ========== phase 2: softmax / combine (overlaps A phase) ==========
        ncopies = max(1, NC // max(1, DPB))
        inv_sqrt_dp = 1.0 / (float(np.sqrt(DP)) * ncopies)
        sr2 = sb.tile([J, M], f32, name="sr2")
        nc.gpsimd.dma_start(sr2[:, :], sred_b[:, :])
        nc.scalar.activation(
            ej[:, :], sr2[:, :], AF.Exp, bias=0.0, scale=inv_sqrt_dp,
        )
        nc.vector.tensor_mul(out=ej[:, :], in0=ej[:, :], in1=vm_sb[:, :])
        # r0 = 1/max(sum_j ej, eps); g0 = 1/sum_k eg; g1 = eg[esel]
        # partition reductions on GpSimd keep the PE/DVE queues clean
        jr = sb.tile([J, M], bf16, tag="jm", bufs=1, name="jr")
        nc.gpsimd.partition_all_reduce(jr[:, :], ej[:, :], J,
                                       bass_isa.ReduceOp.add)
        kr = sb.tile([K, M], bf16, tag="km", bufs=1, name="kr")
        nc.gpsimd.partition_all_reduce(kr[:, :], eg_sb[:, :], K,
                                       bass_isa.ReduceOp.add)
        if not b2z:
            nc.vector.tensor_copy(cwsum_bf[0:1, :], jr[0:1, :])  # rowsum
        nc.vector.tensor_scalar_max(r0[0:1, :], jr[0:1, :], 1e-8)
        with nc.allow_low_precision(reason="bf16 softmax denominators"):
            nc.vector.reciprocal(r0[0:1, :], r0[0:1, :])
            nc.vector.reciprocal(g0[0:1, :], kr[0:1, :])
        nc.vector.tensor_scalar_mul(eg_sb[:, :], eg_sb[:, :],
                                    esel_sb[:, 0:1])
        er = sb.tile([K, M], bf16, tag="km", bufs=1, name="er")
        nc.gpsimd.partition_all_reduce(er[:, :], eg_sb[:, :], K,
                                       bass_isa.ReduceOp.add)
        nc.vector.tensor_mul(out=g1[0:1, :], in0=er[0:1, :], in1=g0[0:1, :])
        # w = gate[e] / rowsum, broadcast to [J, M]; fold into ej in place
        nc.vector.tensor_mul(out=g1[0:1, :], in0=g1[0:1, :], in1=r0[0:1, :])
        if not b2z:
            # cwsum = rowsum * w  (~gate[e], but 0 for pairless masks)
            nc.vector.tensor_mul(out=cwsum_bf[0:1, :], in0=cwsum_bf[0:1, :],
                                 in1=g1[0:1, :])
        nc.gpsimd.partition_broadcast(wbJ[:, :], g1[0:1, :], J)
        nc.vector.tensor_mul(out=ej[:, :], in0=ej[:, :], in1=wbJ[:, :])

        # cwb: broadcast combine-weight rows across partitions on GpSimd
        def cwb_build(mc):
            cwb = cwb_tile(f"cwb{mc}")
            for j in range(J):
                row_t = sb.tile([1, MCW], bf16, tag="cwrow", bufs=2,
                                name="row_t")
                nc.gpsimd.dma_start(
                    row_t[0:1, :], ej[j:j + 1, mc * MCW:(mc + 1) * MCW])
                nc.gpsimd.partition_broadcast(cwb[:, j, :], row_t[0:1, :], 128)
            return cwb

        # ================= steady pipeline: M -> hid -> W2 =================
        def m_phase(mc, mh_t):
            hm_c = hmc_tile(f"hm{mc}")
            nc.sync.dma_start(
                hm_c[:, :, :],
                hmT.ap().rearrange("(o p) m -> p o m", p=128)[
                    :, :, mc * MCW:(mc + 1) * MCW],
            )
            for hg in range(0, HB, HGS):
                psm = [psum_mm(f"psm{gi}") for gi in range(HGS)]
                for dbb in range(0, DB, DSLAB):
                    w_t = w1t_tile("w1b_t")
                    nc.sync.dma_start(
                        w_t[:, :, :],
                        w1b[dbb * 128:(dbb + DSLAB) * 128,
                            hg * 128:(hg + HGS) * 128].rearrange(
                                "(o p) h -> p o h", p=128),
                    )
                    for i in range(DSLAB):
                        db = dbb + i
                        for gi in range(HGS):
                            nc.tensor.matmul(
                                psm[gi][:, :MCW],
                                w_t[:, i, gi * 128:(gi + 1) * 128],
                                hm_c[:, db, :],
                                start=db == 0, stop=db == DB - 1,
                            )
                for gi in range(HGS):
                    hb = hg + gi
                    nc.scalar.activation(
                        mh_t[:, hb, :], psm[gi][:, :MCW], AF.Identity,
                        bias=b1_sb[:, hb:hb + 1], scale=1.0,
                    )

        def hid_phase(mc, mh_t, cwb):
            c0 = mc * MCW
            for hb in range(HB):
                scr = sb.tile([128, MCW], bf16, tag="scr", bufs=2, name="scr")
                for j in range(J):
                    off = int(offs[j])
                    x_t = sb.tile([128, MCW], bf16, tag="xt", bufs=2,
                                  name="x_t")
                    nc.vector.tensor_add(
                        out=x_t[:, :],
                        in0=A1T[:, hb, PAD + off + c0: PAD + off + c0 + MCW],
                        in1=mh_t[:, hb, :],
                    )
                    g_t = sb.tile([128, MCW], bf16, tag="gt", bufs=2,
                                  name="g_t")
                    nc.scalar.activation(
                        g_t[:, :], x_t[:, :], hid_af, bias=0.0, scale=1.0,
                    )
                    if j == 0:
                        dst = scr[:, :] if J > 1 else mh_t[:, hb, :]
                        nc.vector.tensor_mul(
                            out=dst, in0=g_t[:, :], in1=cwb[:, j, :],
                        )
                    elif j < J - 1:
                        nc.vector.tensor_mul(
                            out=g_t[:, :], in0=g_t[:, :], in1=cwb[:, j, :]
                        )
                        nc.vector.tensor_add(
                            out=scr[:, :], in0=scr[:, :], in1=g_t[:, :]
                        )
                    else:
                        nc.vector.tensor_mul(
                            out=g_t[:, :], in0=g_t[:, :], in1=cwb[:, j, :]
                        )
                        nc.vector.tensor_add(
                            out=mh_t[:, hb, :], in0=scr[:, :], in1=g_t[:, :]
                        )

        def w2_pass(mc, mh_t, c0, cw, fire_rs=False):
            lo = c0 - mc * MCW
            for dg in range(0, DB, DGS):
                psd = [psum_mm(f"psd{gi}") for gi in range(DGS)]
                for hbb in range(0, HB, HSLAB):
                    w2_t = w2t_tile("w2_t")
                    nc.sync.dma_start(
                        w2_t[:, :, :],
                        w2[hbb * 128:(hbb + HSLAB) * 128,
                           dg * 128:(dg + DGS) * 128].rearrange(
                               "(o p) d -> p o d", p=128),
                    )
                    for i in range(HSLAB):
                        hb = hbb + i
                        for gi in range(DGS):
                            nc.tensor.matmul(
                                psd[gi][:, :cw],
                                w2_t[:, i, gi * 128:(gi + 1) * 128],
                                mh_t[:, hb, lo:lo + cw],
                                start=hb == 0,
                                stop=(hb == HB - 1) if b2z else False,
                            )
                for gi in range(DGS):
                    db = dg + gi
                    if not b2z:
                        nc.tensor.matmul(
                            psd[gi][:, :cw],
                            b2_sb[0:1, db * 128:(db + 1) * 128],
                            cwsum_bf[0:1, c0:c0 + cw],
                            start=False, stop=True,
                        )
                    d_t = sb.tile([128, MCW], bf16, tag="dt", bufs=2,
                                  name="d_t")
                    nc.scalar.activation(
                        d_t[:, :cw], psd[gi][:, :cw], AF.Identity,
                        bias=0.0, scale=1.0,
                    )
                    band = db // DBPB
                    row0 = (db % DBPB) * 128
                    nc.sync.dma_start(
                        bounce[band][row0:row0 + 128, c0:c0 + cw],
                        d_t[:, :cw],
                    )
                if fire_rs and (dg + DGS) % DBPB == 0:
                    rs_group(dg // DBPB)

        def rs_group(g):
            nc.gpsimd.collective_compute(
                "ReduceScatter",
                mybir.AluOpType.add,
                ins=[bounce[g].opt()],
                outs=[rsout[g].opt()],
                replica_groups=[list(range(NC))],
            )
            nc.gpsimd.dma_start(
                outp[g * (DBAND // NC):(g + 1) * (DBAND // NC), :],
                rsout[g][:, :],
            )

        cwb_store = {mc: cwb_build(mc) for mc in range(NMC)}
        mh_store = {}
        for mc in range(NMC):
            mh_store[mc] = mh_tile(f"mh{mc}")
            m_phase(mc, mh_store[mc])
            hid_phase(mc, mh_store[mc], cwb_store[mc])
            if mc >= 1:
                pm = mc - 1
                w2_pass(pm, mh_store[pm], pm * MCW, MCW,
                        fire_rs=(pm == NMC - 1))
                del mh_store[pm]
        mc = NMC - 1
        w2_pass(mc, mh_store[mc], mc * MCW, MCW, fire_rs=True)

    nc.finalize()
    return nc


def _prepare(inputs, cfg):
    import ml_dtypes
    BF16 = ml_dtypes.bfloat16
    D, H, M, U, DP, K = cfg["D"], cfg["H"], cfg["M"], cfg["U"], cfg["DP"], cfg["K"]
    HB, DPB = H // 128, DP // 128
    offs, valid = cfg["offs"], cfg["valid"]
    J = len(offs)

    h = np.asarray(inputs["h_L"], dtype=np.float32)[0]
    m_idx = np.asarray(inputs["mask_indices"]).astype(np.int64)
    u_idx = np.asarray(inputs["unmasked_indices"]).astype(np.int64)

    hmT = np.ascontiguousarray(h[m_idx].astype(BF16).T)
    huT = np.ascontiguousarray(h[u_idx].astype(BF16).T)
    wq = np.asarray(inputs["Wq"], np.float32).astype(BF16)
    wk = np.asarray(inputs["Wk"], np.float32).astype(BF16)
    wr = np.asarray(inputs["Wr"], np.float32).astype(BF16)
    bq = np.asarray(inputs["bq"], np.float32)
    bk = np.asarray(inputs["bk"], np.float32)
    brc = np.zeros((128, 1), np.float32)
    brc[:K, 0] = np.asarray(inputs["br"], np.float32)
    vm = np.ascontiguousarray(valid).astype(BF16)  # [J, M]

    W1 = np.asarray(inputs["W1"], np.float32)
    W2 = np.asarray(inputs["W2"], np.float32)
    b1 = np.asarray(inputs["b1"], np.float32)
    b2 = np.asarray(inputs["b2"], np.float32)

    DPBT = max(1, DP // 128)
    in_maps = []
    for c in range(cfg["NC"]):
        e = c % K
        dpb = c % DPBT
        dsl = slice(dpb * 128, (dpb + 1) * 128)
        sel = np.zeros((K, 1), np.float32)
        sel[e, 0] = 1.0
        in_maps.append({
            "hmT": hmT, "huT": huT,
            "w1a": np.ascontiguousarray(W1[e][:D]).astype(BF16),
            "w1b": np.ascontiguousarray(W1[e][D:]).astype(BF16),
            "w2": W2[e].astype(BF16),
            "wq": np.ascontiguousarray(wq[:, dsl]),
            "wk": np.ascontiguousarray(wk[:, dsl]),
            "wr": wr,
            "b1c": np.ascontiguousarray(b1[e].reshape(HB, 128).T),
            "b2r": b2[e].reshape(1, D).astype(BF16),
            "bqc": np.ascontiguousarray(bq[dsl].reshape(128, 1)),
            "bkc": np.ascontiguousarray(bk[dsl].reshape(128, 1)),
            "brc": brc,
            "esel": sel, "vmask": vm,
        })
    return in_maps, m_idx


def _run(cfg, in_maps, trace=False, sim=False):
    global LAST_RESULT
    key = cfg["key"]
    if key not in _GRAPH_CACHE:
        _GRAPH_CACHE[key] = build_graph(cfg)
    nc = _GRAPH_CACHE[key]
    if sim:
        from concourse import bass_interp
        s = bass_interp.MultiCoreSim(nc, cfg["NC"])
        for c in range(cfg["NC"]):
            for k, v in in_maps[c].items():
                s.cores[c].tensor(k)[:] = v
        s.simulate(check_with_hw=False)
        return [{"out": np.asarray(s.cores[c].mem_tensor("out"))}
                for c in range(cfg["NC"])]
    from concourse import bass_utils
    kw = {}
    if trace and os.environ.get("KERNEL_TRACE_DIR"):
        kw["tmpdir"] = os.environ["KERNEL_TRACE_DIR"]
    res = bass_utils.run_bass_kernel_spmd(
        nc, in_maps, core_ids=list(range(cfg["NC"])), trace=trace, **kw,
    )
    LAST_RESULT = res
    return res.results


def kernel_impl(inputs, D, K, L, M, U, DP, H, NC, MCW, QCW, NRS, sim=False,
                hid_act="Gelu", SCW=None):
    PMAX = M * 10

    m_idx = np.asarray(inputs["mask_indices"]).astype(np.int64)
    u_idx = np.asarray(inputs["unmasked_indices"]).astype(np.int64)
    r = int(np.asarray(inputs["range_r"]))

    offs, valid = build_tables(m_idx, u_idx, r, PMAX)
    J = len(offs)
    if J == 0:
        return np.zeros((1, L, D), np.float32)
    PAD = int(max(8, np.max(np.abs(offs))))
    PAD = (PAD + 7) // 8 * 8
    b2z = not np.any(np.asarray(inputs["b2"]))

    cfg = {
        "D": D, "H": H, "M": M, "U": U, "DP": DP, "K": K, "NC": NC,
        "offs": offs, "valid": valid, "PAD": PAD, "b2z": b2z,
        "MCW": MCW, "QCW": QCW, "NRS": NRS, "hid_act": hid_act,
        "key": (D, H, M, U, DP, K, NC, MCW, QCW, NRS, PAD, hid_act, b2z,
                tuple(offs.tolist())),
    }

    in_maps, m_idx = _prepare(inputs, cfg)
    results = _run(cfg, in_maps, trace=bool(os.environ.get("KERNEL_TRACE")),
                   sim=sim)

    # cores hold D-band-interleaved rows: core c, band g covers absolute
    # d-rows [g*DBAND + c*BR, g*DBAND + (c+1)*BR)
    DBAND = D // NRS
    BR = DBAND // NC
    deltaT = np.empty((D, M), np.float32)
    for c in range(NC):
        oc = np.asarray(results[c]["out"], np.float32)  # [D//NC, M]
        for g in range(NRS):
            deltaT[g * DBAND + c * BR: g * DBAND + (c + 1) * BR] = \
                oc[g * BR:(g + 1) * BR]
    delta_md = deltaT.T  # [M, D]
    out = np.zeros((L, D), np.float32)
    if len(np.unique(m_idx)) == len(m_idx):
        out[m_idx] = delta_md
    else:
        np.add.at(out, m_idx, delta_md)
    return out[None]


def kernel(**inputs):
    return kernel_impl(
        inputs, D=4096, K=8, L=2048, M=1024, U=1024, DP=512, H=2048,
        NC=NCORES, MCW=512, QCW=512, NRS=4,
    )
